# revision 4
# baseline (speedup 1.0000x reference)
"""Trainium2 Bass kernel for ExemplarGNN2AdjModel (gnn_message_passing).

Math:
  h  = relu(relu(x@W1+b1)@W2+b2)                      # [512,128] node encoder
  scores[i,j] = Wp2 . relu(Wp1a.h_i + Wp1b.h_j + Wp1c.|h_i-h_j| + bp1) + bp2

Device algorithm (per core, SPMD over 8 cores; core c handles 64 rows of i):
  - Each core receives x pre-rolled by c*64 rows and pre-transposed (xT), so the
    identical program computes rows [c*64, c*64+64) in its local (rolled) node
    order; the host un-rolls the output columns afterwards.
  - |h_i-h_j| = h_i + h_j - 2*min(h_i,h_j): the h_i term is folded into the
    per-i bias matrix (wp1a += w3), the h_j term into the B matmul
    (w2p += w3), and the per-pair part is -2*w3^T min(h_i, h_j).
  - Encoder runs on-device in bf16 (all 512 nodes, replicated per core),
    fp32 PSUM accumulation, fp32 biases.  A2 = wp1a^T h + bp1 precomputed once.
  - Per row i (64 iters), engines balanced three ways so the in-order PE
    (3 x 512-col matmuls/row = the true roofline) is never starved:
      d_i  = min(h, h_i)            even rows: DVE tensor_scalar (2x bf16)
                                    odd rows:  GpSimd tensor_scalar (else idle)
      P    = w2p^T h + w3^T d_i     (accumulating PE matmuls into PSUM)
      hid  = relu(P + A2[:,i])      even rows: ACT (bias); odd rows: DVE
                                    (tensor_scalar add,max) -- whole-row ops,
                                    alternating, so no engine exceeds the PE's
                                    ~1280ns/2-row group budget
      out[r,:] += embW_i^T hid      (PE matmul; embW_i = embbuf[:, 63-i : -i]
                                    sliding window puts Wp2 in column i, so a
                                    127-col zero buffer with Wp2 at col 63
                                    replaces the old 1MB embedded-Wp2 DMA)
  - Output PSUM is split in two [32,512] blocks (rows 0-31 / 32-63) so the
    first half's bp2-add + DMA-out overlap the second half's compute.
  - xtp is DMA'd in 5 k-chunks so encoder matmuls start as chunks land; a few
    small dummy matmuls at start keep the PE busy (HAM clock warm) during DMA.
"""

import numpy as np
import ml_dtypes

B = 512
IN_DIM = 595
HID = 128
NCORES = 8
RPC = B // NCORES  # rows per core = 64
DEFER = 4   # rows between producing hid(i) and its out-row matmul
LOOKA = 4   # rows of min-lookahead
N_WARM_MM = 10  # small dummy matmuls to warm the PE HAM clock during DMAs
WARM_N = 256    # free dim of warm matmuls

# in_dim k-tiles for the first encoder matmul (contraction over 595)
KT = [(0, 128), (128, 256), (256, 384), (384, 512), (512, 595)]

_PROGRAM_CACHE = {}


def _build_program():
    import concourse.mybir as mybir
    import concourse.tile as tile
    from concourse import bacc

    f32 = mybir.dt.float32
    bf16 = mybir.dt.bfloat16
    Act = mybir.ActivationFunctionType
    Alu = mybir.AluOpType

    nc = bacc.Bacc("TRN2", target_bir_lowering=False)

    NKT = len(KT)
    xt_d = nc.dram_tensor("xtp", [HID, NKT * B], bf16, kind="ExternalInput")
    w1_d = nc.dram_tensor("w1p", [HID, NKT * HID], bf16, kind="ExternalInput")
    wpack_d = nc.dram_tensor("wpack", [HID, 4 * HID], bf16, kind="ExternalInput")
    bias_d = nc.dram_tensor("biases", [HID, 4], f32, kind="ExternalInput")
    wp2_d = nc.dram_tensor("wp2col", [HID, 1], bf16, kind="ExternalInput")
    out_d = nc.dram_tensor("out", [RPC, B], f32, kind="ExternalOutput")

    HB = RPC // 2  # 32: rows per output half-block

    with tile.TileContext(nc) as tc:
        with (
            tc.tile_pool(name="consts", bufs=1) as consts,
            tc.tile_pool(name="setup", bufs=1) as setup,
            tc.tile_pool(name="dwork", bufs=7) as dwork,
            tc.tile_pool(name="hwork", bufs=7) as hwork,
            tc.tile_pool(name="penc", bufs=1, space="PSUM") as penc,
            tc.tile_pool(name="ppair", bufs=5, space="PSUM") as ppair,
            tc.tile_pool(name="pout", bufs=1, space="PSUM") as pout,
        ):
            # ---- PE warm-up: small dummy matmuls keep the PE busy while the
            # input DMAs land (HAM clock-gate ramps to 2.4 GHz; ACT table /
            # IRAM loads happen off the critical path).
            scratch = setup.tile([HID, B], bf16)
            nc.vector.memset(scratch, 0.0)
            scratch1 = setup.tile([HID, 1], f32)
            nc.scalar.activation(scratch1, scratch[:, 0:1], Act.Relu)
            warmp = penc.tile([HID, B], f32, name="warmp", tag="encp")

            def warm_mm(n, w=WARM_N):
                for _ in range(n):
                    nc.tensor.matmul(
                        warmp[:, 0:w], lhsT=scratch[:, 0:HID], rhs=scratch[:, 0:w],
                        start=True, stop=True, skip_group_check=True,
                    )

            warm_mm(N_WARM_MM)

            # ---- input loads: xtp in 5 k-chunks on the sync ring, weights on
            # the scalar ring.
            xt_all = consts.tile([HID, NKT * B], bf16)
            for k in range(NKT):
                nc.sync.dma_start(
                    out=xt_all[:, k * B : (k + 1) * B],
                    in_=xt_d[:, k * B : (k + 1) * B],
                )
            w1_all = consts.tile([HID, NKT * HID], bf16)
            nc.scalar.dma_start(out=w1_all, in_=w1_d[:, :])
            wpack = consts.tile([HID, 4 * HID], bf16)
            nc.scalar.dma_start(out=wpack, in_=wpack_d[:, :])
            biases = consts.tile([HID, 4], f32)
            nc.scalar.dma_start(out=biases, in_=bias_d[:, :])
            # sliding-window Wp2 buffer: zeros with Wp2 at column 63; the out
            # matmul for row j uses embbuf[:, 63-j+c] == Wp2 iff c == j.
            embbuf = consts.tile([HID, 2 * RPC - 1], bf16)
            nc.vector.memset(embbuf, 0.0)
            nc.scalar.dma_start(out=embbuf[:, RPC - 1 : RPC], in_=wp2_d[:, :])

            xt_sb = [xt_all[:, k * B : (k + 1) * B] for k in range(NKT)]
            w1_sb = [w1_all[:, k * HID : (k + 1) * HID] for k in range(NKT)]
            w2_sb = wpack[:, 0 * HID : 1 * HID]
            wp1a_sb = wpack[:, 1 * HID : 2 * HID]
            w2p_sb = wpack[:, 2 * HID : 3 * HID]
            w3_sb = wpack[:, 3 * HID : 4 * HID]
            b1_sb = biases[:, 0:1]
            b2_sb = biases[:, 1:2]
            bp1_sb = biases[:, 2:3]
            bp2_sb = biases[0:HB, 3:4]

            # ---- encoder: h1 = relu(W1^T xT + b1), hT = relu(W2^T h1 + b2) ----
            h1p = penc.tile([HID, B], f32, name="encp", tag="encp")
            for k in range(len(KT)):
                nc.tensor.matmul(
                    h1p, lhsT=w1_sb[k], rhs=xt_sb[k],
                    start=(k == 0), stop=(k == len(KT) - 1),
                )
            warm_mm(3)  # bridge PE over relu1
            h1bf = setup.tile([HID, B], bf16)
            nc.scalar.activation(h1bf, h1p, Act.Relu, bias=b1_sb)

            h2p = penc.tile([HID, B], f32, name="encp2", tag="encp")
            nc.tensor.matmul(h2p, lhsT=w2_sb, rhs=h1bf, start=True, stop=True)
            warm_mm(3)  # bridge PE over relu2
            # hbf (bf16, ACT) and hT (fp32, DVE) in parallel from the same
            # PSUM tile; hT fp32 is the per-row scalar operand of the min
            # (tensor_scalar scalars must be fp32)
            hbf = setup.tile([HID, B], bf16)
            nc.scalar.activation(hbf, h2p, Act.Relu, bias=b2_sb)
            hT = setup.tile([HID, B], f32)
            nc.vector.tensor_scalar(hT, h2p, b2_sb, 0.0, Alu.add, Alu.max)

            # mins: even rows on DVE, odd rows on GpSimd (otherwise idle)
            def emit_min(j, dtiles):
                if j in dtiles or j >= RPC:
                    return
                d = dwork.tile([HID, B], bf16, name="dtile")
                eng = nc.vector if (j % 2 == 0) else nc.gpsimd
                eng.tensor_scalar(d, hbf, hT[:, j : j + 1], None, Alu.min)
                dtiles[j] = d

            dtiles = {}
            emit_min(0, dtiles)
            emit_min(1, dtiles)

            # ---- A2 = wp1a^T h + bp1  (per-i relu bias columns) ----
            a2p = penc.tile([HID, B], f32, name="encp3", tag="encp")
            nc.tensor.matmul(a2p, lhsT=wp1a_sb, rhs=hbf, start=True, stop=True)
            a2 = setup.tile([HID, B], f32)
            nc.scalar.activation(a2, a2p, Act.Identity, bias=bp1_sb)
            emit_min(2, dtiles)
            emit_min(3, dtiles)
            warm_mm(2)  # bridge PE over the a2 add + first mins

            # ---- pairwise main loop over this core's 64 rows ----
            outp = [pout.tile([HB, B], f32, name=f"outp{_b}") for _b in range(2)]
            outs = [None, None]
            pending = {}

            def emit_out(j):
                hid_j = pending.pop(j)
                blk = j // HB
                jj = j % HB
                nc.tensor.matmul(
                    outp[blk],
                    lhsT=embbuf[:, RPC - 1 - jj : RPC - 1 - jj + HB],
                    rhs=hid_j,
                    start=(jj == 0), stop=(jj == HB - 1),
                    skip_group_check=True,
                )
                if jj == HB - 1:
                    # half-block complete: bias-add + DMA out overlap the rest
                    o = setup.tile([HB, B], f32, name=f"outs{blk}")
                    nc.vector.tensor_scalar(o, outp[blk], bp2_sb, None, Alu.add)
                    nc.sync.dma_start(
                        out=out_d[blk * HB : (blk + 1) * HB, :], in_=o
                    )
                    outs[blk] = o

            G = 2
            for g in range(RPC // G):
                i0 = G * g
                ils = [i0, i0 + 1]
                for il in ils:
                    emit_min(il + LOOKA, dtiles)
                # out matmuls for rows DEFER back (oldest deps first)
                for il in ils:
                    if il - DEFER >= 0:
                        emit_out(il - DEFER)
                pps = []
                for il in ils:
                    pp = ppair.tile([HID, B], f32, name="pp")
                    nc.tensor.matmul(
                        pp, lhsT=w2p_sb, rhs=hbf,
                        start=True, stop=False, skip_group_check=True,
                    )
                    pps.append(pp)
                for il, pp in zip(ils, pps):
                    nc.tensor.matmul(
                        pp, lhsT=w3_sb, rhs=dtiles.pop(il),
                        start=False, stop=True, skip_group_check=True,
                    )
                # whole-row relus, alternating ACT / DVE
                for il, pp in zip(ils, pps):
                    hid = hwork.tile([HID, B], bf16, name="hid")
                    if il % 2 == 0:
                        nc.scalar.activation(
                            hid, pp, Act.Relu, bias=a2[:, il : il + 1]
                        )
                    else:
                        nc.vector.tensor_scalar(
                            hid, pp, a2[:, il : il + 1], 0.0, Alu.add, Alu.max
                        )
                    pending[il] = hid
            for j in range(RPC - DEFER, RPC):
                emit_out(j)

    nc.finalize()
    return nc


def _get_program():
    if "nc" not in _PROGRAM_CACHE:
        _PROGRAM_CACHE["nc"] = _build_program()
    return _PROGRAM_CACHE["nc"]


def _make_in_maps(x, W1, b1, W2, b2, Wp1, bp1, Wp2, bp2):
    bf16 = ml_dtypes.bfloat16
    f32 = np.float32
    x = np.asarray(x, dtype=f32)
    W1 = np.asarray(W1, dtype=f32)
    W2 = np.asarray(W2, dtype=f32)
    Wp1 = np.asarray(Wp1, dtype=f32)
    Wp2 = np.asarray(Wp2, dtype=f32).reshape(HID, 1)
    b1c = np.ascontiguousarray(np.asarray(b1, dtype=f32).reshape(HID, 1))
    b2c = np.ascontiguousarray(np.asarray(b2, dtype=f32).reshape(HID, 1))
    bp1c = np.ascontiguousarray(np.asarray(bp1, dtype=f32).reshape(HID, 1))
    bp2c = np.full((RPC // 2, 1), np.asarray(bp2, dtype=f32).reshape(-1)[0], dtype=f32)

    # |h_i - h_j| = h_i + h_j - 2*min(h_i, h_j) folds (see module docstring)
    w3f = Wp1[2 * HID : 3 * HID, :]
    wp1a = Wp1[0:HID, :] + w3f
    w2p = Wp1[HID : 2 * HID, :] + w3f
    w3 = -2.0 * w3f

    NKT = len(KT)
    KPAD = NKT * HID  # 640: in_dim padded so every k-tile is 128 partitions

    # packed weights [w2 | wp1a | w2p | w3] and biases [b1 | b2 | bp1 | bp2col]
    wpack = np.concatenate([W2, wp1a, w2p, w3], axis=1).astype(bf16)
    biases = np.zeros((HID, 4), dtype=f32)
    biases[:, 0:1] = b1c
    biases[:, 1:2] = b2c
    biases[:, 2:3] = bp1c
    biases[0 : RPC // 2, 3:4] = bp2c

    wp2col = np.ascontiguousarray(Wp2).astype(bf16)

    # w1 padded to [640, 128], viewed as [128, 5*128]
    w1_pad = np.zeros((KPAD, HID), dtype=f32)
    w1_pad[:IN_DIM] = np.asarray(W1, dtype=f32)
    w1p = np.ascontiguousarray(
        w1_pad.reshape(NKT, HID, HID).transpose(1, 0, 2).reshape(HID, NKT * HID)
    ).astype(bf16)

    shared = dict(w1p=w1p, wpack=wpack, biases=biases, wp2col=wp2col)
    in_maps = []
    for c in range(NCORES):
        xr = np.roll(x, -c * RPC, axis=0)
        xt_pad = np.zeros((KPAD, B), dtype=f32)
        xt_pad[:IN_DIM] = xr.T
        xtp = np.ascontiguousarray(
            xt_pad.reshape(NKT, HID, B).transpose(1, 0, 2).reshape(HID, NKT * B)
        ).astype(bf16)
        m = dict(shared)
        m["xtp"] = xtp
        in_maps.append(m)
    return in_maps


def _run(in_maps, trace=False):
    from concourse.bass_utils import run_bass_kernel_spmd

    nc = _get_program()
    return run_bass_kernel_spmd(
        nc, in_maps, core_ids=list(range(NCORES)), trace=trace
    )


def kernel(x, W1, b1, W2, b2, Wp1, bp1, Wp2, bp2):
    in_maps = _make_in_maps(x, W1, b1, W2, b2, Wp1, bp1, Wp2, bp2)
    res = _run(in_maps, trace=False)
    out = np.empty((B, B), dtype=np.float32)
    for c in range(NCORES):
        blk = np.asarray(res.results[c]["out"], dtype=np.float32)
        out[c * RPC : (c + 1) * RPC, :] = np.roll(blk, c * RPC, axis=1)
    return out


# revision 7
# speedup vs baseline: 3.6793x; 3.6793x over previous
"""Trainium2 Bass kernel for ExemplarGNN2AdjModel (gnn_message_passing).

Math:
  h  = relu(relu(x@W1+b1)@W2+b2)                      # [512,128] node encoder
  scores[i,j] = Wp2 . relu(Wp1a.h_i + Wp1b.h_j + Wp1c.|h_i-h_j| + bp1) + bp2

Device algorithm (per core, SPMD over 8 cores; core c handles 64 rows of i):
  - Each core receives x pre-rolled by c*64 rows and pre-transposed (xT), so the
    identical program computes rows [c*64, c*64+64) in its local (rolled) node
    order; the host un-rolls the output columns afterwards.
  - |h_i-h_j| = h_i + h_j - 2*min(h_i,h_j): the h_i term is folded into the
    per-i bias matrix (wp1a += w3), the h_j term into the B matmul
    (w2p += w3), and the per-pair part is -2*w3^T min(h_i, h_j).
  - Encoder runs on-device in bf16 (all 512 nodes, replicated per core),
    fp32 PSUM accumulation, fp32 biases.  A2 = wp1a^T h + bp1 precomputed once.
  - Per row i (64 iters), engines balanced three ways so the in-order PE
    (3 x 512-col matmuls/row = the true roofline) is never starved:
      d_i  = min(h, h_i)            even rows: DVE tensor_scalar (2x bf16)
                                    odd rows:  GpSimd tensor_scalar (else idle)
      P    = w2p^T h + w3^T d_i     (accumulating PE matmuls into PSUM)
      hid  = relu(P + A2[:,i])      even rows: ACT (bias); odd rows: DVE
                                    (tensor_scalar add,max) -- whole-row ops,
                                    alternating, so no engine exceeds the PE's
                                    ~1280ns/2-row group budget
      out[r,:] += embW_i^T hid      (PE matmul; embW_i = embbuf[:, 63-i : -i]
                                    sliding window puts Wp2 in column i, so a
                                    127-col zero buffer with Wp2 at col 63
                                    replaces the old 1MB embedded-Wp2 DMA)
  - Output PSUM is split in two [32,512] blocks (rows 0-31 / 32-63) so the
    first half's bp2-add + DMA-out overlap the second half's compute.
  - xtp is DMA'd in 5 k-chunks so encoder matmuls start as chunks land; a few
    small dummy matmuls at start keep the PE busy (HAM clock warm) during DMA.
"""

import numpy as np
import ml_dtypes

B = 512
IN_DIM = 595
HID = 128
NCORES = 8
RPC = B // NCORES  # rows per core = 64
DEFER = 4   # rows between producing hid(i) and its out-row matmul
LOOKA = 4   # rows of min-lookahead
SPLIT = 384  # relu column split: ACT does [0:SPLIT), DVE does [SPLIT:B)
N_WARM_MM = 10  # small dummy matmuls to warm the PE HAM clock during DMAs
WARM_N = 256    # free dim of warm matmuls

# in_dim k-tiles for the first encoder matmul (contraction over 595)
KT = [(0, 128), (128, 256), (256, 384), (384, 512), (512, 595)]

_PROGRAM_CACHE = {}


def _build_program():
    import concourse.mybir as mybir
    import concourse.tile as tile
    from concourse import bacc

    f32 = mybir.dt.float32
    bf16 = mybir.dt.bfloat16
    Act = mybir.ActivationFunctionType
    Alu = mybir.AluOpType

    nc = bacc.Bacc("TRN2", target_bir_lowering=False)

    NKT = len(KT)
    xt_d = nc.dram_tensor("xtp", [HID, NKT * B], bf16, kind="ExternalInput")
    w1_d = nc.dram_tensor("w1p", [HID, NKT * HID], bf16, kind="ExternalInput")
    wpack_d = nc.dram_tensor("wpack", [HID, 4 * HID], bf16, kind="ExternalInput")
    bias_d = nc.dram_tensor("biases", [HID, 4], f32, kind="ExternalInput")
    wp2_d = nc.dram_tensor("wp2col", [HID, 1], bf16, kind="ExternalInput")
    out_d = nc.dram_tensor("out", [RPC, B], f32, kind="ExternalOutput")

    HB = RPC // 2  # 32: rows per output half-block

    with tile.TileContext(nc) as tc:
        with (
            tc.tile_pool(name="consts", bufs=1) as consts,
            tc.tile_pool(name="setup", bufs=1) as setup,
            tc.tile_pool(name="dwork", bufs=7) as dwork,
            tc.tile_pool(name="hwork", bufs=7) as hwork,
            tc.tile_pool(name="penc", bufs=1, space="PSUM") as penc,
            tc.tile_pool(name="ppair", bufs=5, space="PSUM") as ppair,
            tc.tile_pool(name="pout", bufs=1, space="PSUM") as pout,
        ):
            # ---- PE warm-up: small dummy matmuls keep the PE busy while the
            # input DMAs land (HAM clock-gate ramps to 2.4 GHz; ACT table /
            # IRAM loads happen off the critical path).
            scratch = setup.tile([HID, B], bf16)
            nc.vector.memset(scratch, 0.0)
            scratch1 = setup.tile([HID, 1], f32)
            nc.scalar.activation(scratch1, scratch[:, 0:1], Act.Relu)
            warmp = penc.tile([HID, B], f32, name="warmp", tag="encp")

            def warm_mm(n, w=WARM_N):
                for _ in range(n):
                    nc.tensor.matmul(
                        warmp[:, 0:w], lhsT=scratch[:, 0:HID], rhs=scratch[:, 0:w],
                        start=True, stop=True, skip_group_check=True,
                    )

            warm_mm(N_WARM_MM)

            # ---- input loads: xtp in 5 k-chunks on the sync ring, weights on
            # the scalar ring.
            xt_all = consts.tile([HID, NKT * B], bf16)
            for k in range(NKT):
                nc.sync.dma_start(
                    out=xt_all[:, k * B : (k + 1) * B],
                    in_=xt_d[:, k * B : (k + 1) * B],
                )
            w1_all = consts.tile([HID, NKT * HID], bf16)
            nc.scalar.dma_start(out=w1_all, in_=w1_d[:, :])
            wpack = consts.tile([HID, 4 * HID], bf16)
            nc.scalar.dma_start(out=wpack, in_=wpack_d[:, :])
            biases = consts.tile([HID, 4], f32)
            nc.scalar.dma_start(out=biases, in_=bias_d[:, :])
            # sliding-window Wp2 buffer: zeros with Wp2 at column 63; the out
            # matmul for row j uses embbuf[:, 63-j+c] == Wp2 iff c == j.
            embbuf = consts.tile([HID, 2 * RPC - 1], bf16)
            nc.vector.memset(embbuf, 0.0)
            nc.scalar.dma_start(out=embbuf[:, RPC - 1 : RPC], in_=wp2_d[:, :])

            xt_sb = [xt_all[:, k * B : (k + 1) * B] for k in range(NKT)]
            w1_sb = [w1_all[:, k * HID : (k + 1) * HID] for k in range(NKT)]
            w2_sb = wpack[:, 0 * HID : 1 * HID]
            wp1a_sb = wpack[:, 1 * HID : 2 * HID]
            w2p_sb = wpack[:, 2 * HID : 3 * HID]
            w3_sb = wpack[:, 3 * HID : 4 * HID]
            b1_sb = biases[:, 0:1]
            b2_sb = biases[:, 1:2]
            bp1_sb = biases[:, 2:3]
            bp2_sb = biases[0:HB, 3:4]

            # ---- encoder: h1 = relu(W1^T xT + b1), hT = relu(W2^T h1 + b2) ----
            h1p = penc.tile([HID, B], f32, name="encp", tag="encp")
            for k in range(len(KT)):
                nc.tensor.matmul(
                    h1p, lhsT=w1_sb[k], rhs=xt_sb[k],
                    start=(k == 0), stop=(k == len(KT) - 1),
                )
            warm_mm(3)  # bridge PE over relu1
            h1bf = setup.tile([HID, B], bf16)
            nc.scalar.activation(h1bf, h1p, Act.Relu, bias=b1_sb)

            h2p = penc.tile([HID, B], f32, name="encp2", tag="encp")
            nc.tensor.matmul(h2p, lhsT=w2_sb, rhs=h1bf, start=True, stop=True)
            warm_mm(3)  # bridge PE over relu2
            # hbf (bf16, ACT) and hT (fp32, DVE) in parallel from the same
            # PSUM tile; hT fp32 is the per-row scalar operand of the min
            # (tensor_scalar scalars must be fp32)
            hbf = setup.tile([HID, B], bf16)
            nc.scalar.activation(hbf, h2p, Act.Relu, bias=b2_sb)
            hT = setup.tile([HID, B], f32)
            nc.vector.tensor_scalar(hT, h2p, b2_sb, 0.0, Alu.add, Alu.max)

            # mins on DVE (GpSimd tensor_scalar measured ~7.5us/op and its
            # SBUF-port sharing with the DVE stretches DVE ops ~10x -- unusable)
            def emit_min(j, dtiles):
                if j in dtiles or j >= RPC:
                    return
                d = dwork.tile([HID, B], bf16, name="dtile")
                nc.vector.tensor_scalar(d, hbf, hT[:, j : j + 1], None, Alu.min)
                dtiles[j] = d

            dtiles = {}
            emit_min(0, dtiles)
            emit_min(1, dtiles)

            # ---- A2 = wp1a^T h + bp1  (per-i relu bias columns) ----
            a2p = penc.tile([HID, B], f32, name="encp3", tag="encp")
            nc.tensor.matmul(a2p, lhsT=wp1a_sb, rhs=hbf, start=True, stop=True)
            a2 = setup.tile([HID, B], f32)
            nc.scalar.activation(a2, a2p, Act.Identity, bias=bp1_sb)
            emit_min(2, dtiles)
            emit_min(3, dtiles)
            warm_mm(2)  # bridge PE over the a2 add + first mins

            # ---- pairwise main loop over this core's 64 rows ----
            outp = [pout.tile([HB, B], f32, name=f"outp{_b}") for _b in range(2)]
            outs = [None, None]
            pending = {}

            def emit_out(j):
                hid_j = pending.pop(j)
                blk = j // HB
                jj = j % HB
                nc.tensor.matmul(
                    outp[blk],
                    lhsT=embbuf[:, RPC - 1 - jj : RPC - 1 - jj + HB],
                    rhs=hid_j,
                    start=(jj == 0), stop=(jj == HB - 1),
                    skip_group_check=True,
                )
                if jj == HB - 1:
                    # half-block complete: bias-add + DMA out overlap the rest
                    o = setup.tile([HB, B], f32, name=f"outs{blk}")
                    nc.vector.tensor_scalar(o, outp[blk], bp2_sb, None, Alu.add)
                    nc.sync.dma_start(
                        out=out_d[blk * HB : (blk + 1) * HB, :], in_=o
                    )
                    outs[blk] = o

            G = 2
            for g in range(RPC // G):
                i0 = G * g
                ils = [i0, i0 + 1]
                for il in ils:
                    emit_min(il + LOOKA, dtiles)
                # out matmuls for rows DEFER back (oldest deps first)
                for il in ils:
                    if il - DEFER >= 0:
                        emit_out(il - DEFER)
                pps = []
                for il in ils:
                    pp = ppair.tile([HID, B], f32, name="pp")
                    nc.tensor.matmul(
                        pp, lhsT=w2p_sb, rhs=hbf,
                        start=True, stop=False, skip_group_check=True,
                    )
                    pps.append(pp)
                for il, pp in zip(ils, pps):
                    nc.tensor.matmul(
                        pp, lhsT=w3_sb, rhs=dtiles.pop(il),
                        start=False, stop=True, skip_group_check=True,
                    )
                # relu split: ACT does cols [0:SPLIT), DVE the rest
                for il, pp in zip(ils, pps):
                    hid = hwork.tile([HID, B], bf16, name="hid")
                    nc.scalar.activation(
                        hid[:, 0:SPLIT], pp[:, 0:SPLIT], Act.Relu,
                        bias=a2[:, il : il + 1],
                    )
                    nc.vector.tensor_scalar(
                        hid[:, SPLIT:B], pp[:, SPLIT:B],
                        a2[:, il : il + 1], 0.0, Alu.add, Alu.max,
                    )
                    pending[il] = hid
            for j in range(RPC - DEFER, RPC):
                emit_out(j)

    nc.finalize()
    return nc


def _get_program():
    if "nc" not in _PROGRAM_CACHE:
        _PROGRAM_CACHE["nc"] = _build_program()
    return _PROGRAM_CACHE["nc"]


def _make_in_maps(x, W1, b1, W2, b2, Wp1, bp1, Wp2, bp2):
    bf16 = ml_dtypes.bfloat16
    f32 = np.float32
    x = np.asarray(x, dtype=f32)
    W1 = np.asarray(W1, dtype=f32)
    W2 = np.asarray(W2, dtype=f32)
    Wp1 = np.asarray(Wp1, dtype=f32)
    Wp2 = np.asarray(Wp2, dtype=f32).reshape(HID, 1)
    b1c = np.ascontiguousarray(np.asarray(b1, dtype=f32).reshape(HID, 1))
    b2c = np.ascontiguousarray(np.asarray(b2, dtype=f32).reshape(HID, 1))
    bp1c = np.ascontiguousarray(np.asarray(bp1, dtype=f32).reshape(HID, 1))
    bp2c = np.full((RPC // 2, 1), np.asarray(bp2, dtype=f32).reshape(-1)[0], dtype=f32)

    # |h_i - h_j| = h_i + h_j - 2*min(h_i, h_j) folds (see module docstring)
    w3f = Wp1[2 * HID : 3 * HID, :]
    wp1a = Wp1[0:HID, :] + w3f
    w2p = Wp1[HID : 2 * HID, :] + w3f
    w3 = -2.0 * w3f

    NKT = len(KT)
    KPAD = NKT * HID  # 640: in_dim padded so every k-tile is 128 partitions

    # packed weights [w2 | wp1a | w2p | w3] and biases [b1 | b2 | bp1 | bp2col]
    wpack = np.concatenate([W2, wp1a, w2p, w3], axis=1).astype(bf16)
    biases = np.zeros((HID, 4), dtype=f32)
    biases[:, 0:1] = b1c
    biases[:, 1:2] = b2c
    biases[:, 2:3] = bp1c
    biases[0 : RPC // 2, 3:4] = bp2c

    wp2col = np.ascontiguousarray(Wp2).astype(bf16)

    # w1 padded to [640, 128], viewed as [128, 5*128]
    w1_pad = np.zeros((KPAD, HID), dtype=f32)
    w1_pad[:IN_DIM] = np.asarray(W1, dtype=f32)
    w1p = np.ascontiguousarray(
        w1_pad.reshape(NKT, HID, HID).transpose(1, 0, 2).reshape(HID, NKT * HID)
    ).astype(bf16)

    shared = dict(w1p=w1p, wpack=wpack, biases=biases, wp2col=wp2col)
    in_maps = []
    for c in range(NCORES):
        xr = np.roll(x, -c * RPC, axis=0)
        xt_pad = np.zeros((KPAD, B), dtype=f32)
        xt_pad[:IN_DIM] = xr.T
        xtp = np.ascontiguousarray(
            xt_pad.reshape(NKT, HID, B).transpose(1, 0, 2).reshape(HID, NKT * B)
        ).astype(bf16)
        m = dict(shared)
        m["xtp"] = xtp
        in_maps.append(m)
    return in_maps


def _run(in_maps, trace=False):
    from concourse.bass_utils import run_bass_kernel_spmd

    nc = _get_program()
    return run_bass_kernel_spmd(
        nc, in_maps, core_ids=list(range(NCORES)), trace=trace
    )


def kernel(x, W1, b1, W2, b2, Wp1, bp1, Wp2, bp2):
    in_maps = _make_in_maps(x, W1, b1, W2, b2, Wp1, bp1, Wp2, bp2)
    res = _run(in_maps, trace=False)
    out = np.empty((B, B), dtype=np.float32)
    for c in range(NCORES):
        blk = np.asarray(res.results[c]["out"], dtype=np.float32)
        out[c * RPC : (c + 1) * RPC, :] = np.roll(blk, c * RPC, axis=1)
    return out


# revision 14
# speedup vs baseline: 3.8074x; 1.0348x over previous
"""Trainium2 Bass kernel for ExemplarGNN2AdjModel (gnn_message_passing).

Math:
  h  = relu(relu(x@W1+b1)@W2+b2)                      # [512,128] node encoder
  scores[i,j] = Wp2 . relu(Wp1a.h_i + Wp1b.h_j + Wp1c.|h_i-h_j| + bp1) + bp2

Device algorithm (per core, SPMD over 8 cores; core c handles 64 rows of i):
  - Each core receives x pre-rolled by c*64 rows and pre-transposed (xT), so the
    identical program computes rows [c*64, c*64+64) in its local (rolled) node
    order; the host un-rolls the output columns afterwards.
  - |h_i-h_j| = h_i + h_j - 2*min(h_i,h_j): the h_i term is folded into the
    per-i bias matrix (wp1a += w3), the h_j term into the B matmul
    (w2p += w3), and the per-pair part is -2*w3^T min(h_i, h_j).
  - Encoder runs on-device in bf16 (all 512 nodes, replicated per core),
    fp32 PSUM accumulation, fp32 biases.  A2 = wp1a^T h + bp1 precomputed once.
  - The 64 rows are processed in 16 groups of 4, one row from each of the four
    16-row output blocks (i, i+16, i+32, i+48).  Per group:
      d_r  = min(h, h_r)                  DVE tensor_scalar, 1 group lookahead
      P_r  = w2p^T h + w3^T d_r           8 PE matmuls (acc pairs 4 slots apart
                                          so the same-bank accumulate never
                                          stalls on the PSUM drain)
      hid_r = relu(P_r + A2[:,r])         blocks 0-2 on ACT (bias), block 3 on
                                          DVE (tensor_scalar add,max)
      out[16b+i,:] += embW_r^T hid_r      4 col-tiled PE matmuls to PSUM
                                          partitions 0-15/32-47/64-79/96-111 of
                                          ONE bank -- disjoint col_grp strips
                                          run concurrently (~1 slot for all 4)
    The out matmuls of group g are issued in group g+2 so the in-order PE never
    waits on a relu.
  - embW_r = embbuf[:, 15-i : 31-i]: a sliding window over a 31-column zero
    buffer with Wp2 at column 15 puts Wp2 exactly in stationary column i.
  - Startup: xtp is DMA'd in 5 k-chunks with doorbells spread across the sync/
    gpsimd/vector queues (doorbells cost ~600ns each and serialize per queue);
    encoder matmuls start as chunks land; small dummy matmuls cover the DMA
    window so the PE HAM clock-gate is warm (2.4 GHz) for the steady state.
  - Output: one bias add (bp2) over the four block slices + 4 parallel DMAs.
"""

import numpy as np
import ml_dtypes

B = 512
IN_DIM = 595
HID = 128
NCORES = 8
RPC = B // NCORES  # rows per core = 64
NBLK = 4           # output col-tile blocks
BLK = RPC // NBLK  # 16 rows per block
DEFER_G = 2        # groups between producing hid and its out matmul
LOOKA_G = 0        # extra groups of min lookahead beyond the next group
N_WARM_MM = 7      # dummy matmuls to warm the PE HAM clock during input DMAs
WARM_N = 256       # free dim of warm matmuls

# in_dim k-tiles for the first encoder matmul (contraction over 595)
KT = [(0, 128), (128, 256), (256, 384), (384, 512), (512, 595)]

_PROGRAM_CACHE = {}


def _build_program():
    import concourse.mybir as mybir
    import concourse.tile as tile
    from concourse import bacc

    f32 = mybir.dt.float32
    bf16 = mybir.dt.bfloat16
    Act = mybir.ActivationFunctionType
    Alu = mybir.AluOpType

    nc = bacc.Bacc("TRN2", target_bir_lowering=False)

    NKT = len(KT)
    xt_d = nc.dram_tensor("xtp", [HID, NKT * B], bf16, kind="ExternalInput")
    w1_d = nc.dram_tensor("w1p", [HID, NKT * HID], bf16, kind="ExternalInput")
    wpack_d = nc.dram_tensor("wpack", [HID, 4 * HID], bf16, kind="ExternalInput")
    bias_d = nc.dram_tensor("biases", [HID, 4], f32, kind="ExternalInput")
    wp2_d = nc.dram_tensor("wp2col", [HID, 1], bf16, kind="ExternalInput")
    out_d = nc.dram_tensor("out", [RPC, B], f32, kind="ExternalOutput")

    with tile.TileContext(nc) as tc:
        with (
            tc.tile_pool(name="consts", bufs=1) as consts,
            tc.tile_pool(name="setup", bufs=1) as setup,
            tc.tile_pool(name="dwork", bufs=9) as dwork,
            tc.tile_pool(name="hwork", bufs=13) as hwork,
            tc.tile_pool(name="penc", bufs=1, space="PSUM") as penc,
            tc.tile_pool(name="ppair", bufs=7, space="PSUM") as ppair,
        ):
            # ---- input loads first: doorbells cost ~600ns each and serialize
            # per queue, so spread the xtp chunks across three idle queues.
            xt_all = consts.tile([HID, NKT * B], bf16)
            qeng = [nc.sync, nc.gpsimd, nc.sync, nc.gpsimd, nc.sync]
            for k in range(NKT):
                qeng[k].dma_start(
                    out=xt_all[:, k * B : (k + 1) * B],
                    in_=xt_d[:, k * B : (k + 1) * B],
                )
            w1_all = consts.tile([HID, NKT * HID], bf16)
            nc.scalar.dma_start(out=w1_all, in_=w1_d[:, :])
            wpack = consts.tile([HID, 4 * HID], bf16)
            nc.scalar.dma_start(out=wpack, in_=wpack_d[:, :])
            biases = consts.tile([HID, 4], f32)
            nc.scalar.dma_start(out=biases, in_=bias_d[:, :])

            # ---- PE warm-up over the DMA window (HAM ramps to 2.4 GHz)
            scratch = setup.tile([HID, B], bf16)
            nc.vector.memset(scratch, 0.0)
            scratch1 = setup.tile([HID, 1], f32)
            nc.scalar.activation(scratch1, scratch[:, 0:1], Act.Relu)
            warmp = penc.tile([HID, B], f32, name="warmp", tag="encp")

            def warm_mm(n, w=WARM_N):
                for _ in range(n):
                    nc.tensor.matmul(
                        warmp[:, 0:w], lhsT=scratch[:, 0:HID], rhs=scratch[:, 0:w],
                        start=True, stop=True, skip_group_check=True,
                    )

            warm_mm(N_WARM_MM)

            # sliding-window Wp2 buffer: zeros with Wp2 at column BLK-1; the
            # out matmul for block-row i uses embbuf[:, BLK-1-i+c] == Wp2 iff
            # c == i.
            embbuf = consts.tile([HID, 2 * BLK - 1], bf16)
            nc.vector.memset(embbuf, 0.0)
            nc.gpsimd.dma_start(out=embbuf[:, BLK - 1 : BLK], in_=wp2_d[:, :])

            xt_sb = [xt_all[:, k * B : (k + 1) * B] for k in range(NKT)]
            w1_sb = [w1_all[:, k * HID : (k + 1) * HID] for k in range(NKT)]
            w2_sb = wpack[:, 0 * HID : 1 * HID]
            wp1a_sb = wpack[:, 1 * HID : 2 * HID]
            w2p_sb = wpack[:, 2 * HID : 3 * HID]
            w3_sb = wpack[:, 3 * HID : 4 * HID]
            b1_sb = biases[:, 0:1]
            b2_sb = biases[:, 1:2]
            bp1_sb = biases[:, 2:3]
            bp2_sb = biases[:, 3:4]

            # ---- encoder: h1 = relu(W1^T xT + b1), hT = relu(W2^T h1 + b2) ----
            h1p = penc.tile([HID, B], f32, name="encp", tag="encp")
            for k in range(len(KT)):
                nc.tensor.matmul(
                    h1p, lhsT=w1_sb[k], rhs=xt_sb[k],
                    start=(k == 0), stop=(k == len(KT) - 1),
                )
            warm_mm(2)  # bridge PE over relu1
            h1bf = setup.tile([HID, B], bf16)
            nc.scalar.activation(h1bf, h1p, Act.Relu, bias=b1_sb)

            h2p = penc.tile([HID, B], f32, name="encp2", tag="encp")
            nc.tensor.matmul(h2p, lhsT=w2_sb, rhs=h1bf, start=True, stop=True)
            warm_mm(2)  # bridge PE over relu2
            # hbf (bf16, ACT) and hT (fp32, DVE) in parallel from the same
            # PSUM tile; hT fp32 is the per-row scalar operand of the min
            # (tensor_scalar scalars must be fp32)
            hbf = setup.tile([HID, B], bf16)
            nc.scalar.activation(hbf, h2p, Act.Relu, bias=b2_sb)
            hT = setup.tile([HID, B], f32)
            nc.vector.tensor_scalar(hT, h2p, b2_sb, 0.0, Alu.add, Alu.max)

            def emit_min(j, dtiles):
                if j in dtiles or j >= RPC:
                    return
                d = dwork.tile([HID, B], bf16, name="dtile")
                nc.vector.tensor_scalar(d, hbf, hT[:, j : j + 1], None, Alu.min)
                dtiles[j] = d

            # ---- A2 = wp1a^T h + bp1  (per-i relu bias columns) ----
            a2p = penc.tile([HID, B], f32, name="encp3", tag="encp")
            nc.tensor.matmul(a2p, lhsT=wp1a_sb, rhs=hbf, start=True, stop=True)
            a2 = setup.tile([HID, B], f32)
            nc.scalar.activation(a2, a2p, Act.Identity, bias=bp1_sb)

            # out accumulator: ONE PSUM bank; the 4 blocks live at partitions
            # 0-15 / 32-47 / 64-79 / 96-111 so their out matmuls hit disjoint
            # col_grp strips of the PE array and run concurrently.  Reuses the
            # encoder bank (encoder is done before the first out matmul).
            outp = penc.tile([HID, B], f32, name="outp", tag="encp")

            def rows_of(g):
                return [g + BLK * b for b in range(NBLK)] if 0 <= g < BLK else []

            dtiles = {}
            pending = {}

            def emit_outs(g):
                for b in range(NBLK):
                    r = g + BLK * b
                    hid_r = pending.pop(r)
                    nc.tensor.matmul(
                        outp[32 * b : 32 * b + BLK, :],
                        lhsT=embbuf[:, BLK - 1 - g : 2 * BLK - 1 - g],
                        rhs=hid_r,
                        start=(g == 0), stop=(g == BLK - 1),
                        skip_group_check=True,
                        tile_position=(0, 32 * b),
                    )

            # prime the min pipeline
            for g0 in range(LOOKA_G + 1):
                for r in rows_of(g0):
                    emit_min(r, dtiles)

            # ---- pairwise main loop: 16 groups of 4 rows ----
            for g in range(BLK):
                rows = rows_of(g)
                for r in rows_of(g + LOOKA_G + 1):
                    emit_min(r, dtiles)
                # deferred out matmuls (4 col-tiled, concurrent)
                if g - DEFER_G >= 0:
                    emit_outs(g - DEFER_G)
                pps = []
                for r in rows:
                    pp = ppair.tile([HID, B], f32, name="pp")
                    nc.tensor.matmul(
                        pp, lhsT=w2p_sb, rhs=hbf,
                        start=True, stop=False, skip_group_check=True,
                    )
                    pps.append(pp)
                for r, pp in zip(rows, pps):
                    nc.tensor.matmul(
                        pp, lhsT=w3_sb, rhs=dtiles.pop(r),
                        start=False, stop=True, skip_group_check=True,
                    )
                # relus: blocks 0-2 on ACT, block 3 on DVE
                for bi, (r, pp) in enumerate(zip(rows, pps)):
                    hid = hwork.tile([HID, B], bf16, name="hid")
                    if bi < 3:
                        nc.scalar.activation(
                            hid, pp, Act.Relu, bias=a2[:, r : r + 1]
                        )
                    else:
                        nc.vector.tensor_scalar(
                            hid, pp, a2[:, r : r + 1], 0.0, Alu.add, Alu.max
                        )
                    pending[r] = hid
            for g in range(BLK - DEFER_G, BLK):
                emit_outs(g)

            # ---- bp2 add + 4 parallel output DMAs (one per block) ----
            outs = setup.tile([HID, B], f32)
            nc.vector.tensor_scalar(outs, outp, bp2_sb, None, Alu.add)
            dqeng = [nc.sync, nc.gpsimd, nc.scalar, nc.sync]
            for b in range(NBLK):
                dqeng[b].dma_start(
                    out=out_d[BLK * b : BLK * (b + 1), :],
                    in_=outs[32 * b : 32 * b + BLK, :],
                )

    nc.finalize()
    return nc


def _get_program():
    if "nc" not in _PROGRAM_CACHE:
        _PROGRAM_CACHE["nc"] = _build_program()
    return _PROGRAM_CACHE["nc"]


def _make_in_maps(x, W1, b1, W2, b2, Wp1, bp1, Wp2, bp2):
    bf16 = ml_dtypes.bfloat16
    f32 = np.float32
    x = np.asarray(x, dtype=f32)
    W1 = np.asarray(W1, dtype=f32)
    W2 = np.asarray(W2, dtype=f32)
    Wp1 = np.asarray(Wp1, dtype=f32)
    Wp2 = np.asarray(Wp2, dtype=f32).reshape(HID, 1)
    b1c = np.ascontiguousarray(np.asarray(b1, dtype=f32).reshape(HID, 1))
    b2c = np.ascontiguousarray(np.asarray(b2, dtype=f32).reshape(HID, 1))
    bp1c = np.ascontiguousarray(np.asarray(bp1, dtype=f32).reshape(HID, 1))

    # |h_i - h_j| = h_i + h_j - 2*min(h_i, h_j) folds (see module docstring)
    w3f = Wp1[2 * HID : 3 * HID, :]
    wp1a = Wp1[0:HID, :] + w3f
    w2p = Wp1[HID : 2 * HID, :] + w3f
    w3 = -2.0 * w3f

    NKT = len(KT)
    KPAD = NKT * HID  # 640: in_dim padded so every k-tile is 128 partitions

    # packed weights [w2 | wp1a | w2p | w3] and biases [b1 | b2 | bp1 | bp2]
    wpack = np.concatenate([W2, wp1a, w2p, w3], axis=1).astype(bf16)
    biases = np.zeros((HID, 4), dtype=f32)
    biases[:, 0:1] = b1c
    biases[:, 1:2] = b2c
    biases[:, 2:3] = bp1c
    biases[:, 3] = np.asarray(bp2, dtype=f32).reshape(-1)[0]

    wp2col = np.ascontiguousarray(Wp2).astype(bf16)

    # w1 padded to [640, 128], viewed as [128, 5*128]
    w1_pad = np.zeros((KPAD, HID), dtype=f32)
    w1_pad[:IN_DIM] = np.asarray(W1, dtype=f32)
    w1p = np.ascontiguousarray(
        w1_pad.reshape(NKT, HID, HID).transpose(1, 0, 2).reshape(HID, NKT * HID)
    ).astype(bf16)

    shared = dict(w1p=w1p, wpack=wpack, biases=biases, wp2col=wp2col)
    in_maps = []
    for c in range(NCORES):
        xr = np.roll(x, -c * RPC, axis=0)
        xt_pad = np.zeros((KPAD, B), dtype=f32)
        xt_pad[:IN_DIM] = xr.T
        xtp = np.ascontiguousarray(
            xt_pad.reshape(NKT, HID, B).transpose(1, 0, 2).reshape(HID, NKT * B)
        ).astype(bf16)
        m = dict(shared)
        m["xtp"] = xtp
        in_maps.append(m)
    return in_maps


def _run(in_maps, trace=False):
    from concourse.bass_utils import run_bass_kernel_spmd

    nc = _get_program()
    return run_bass_kernel_spmd(
        nc, in_maps, core_ids=list(range(NCORES)), trace=trace
    )


def kernel(x, W1, b1, W2, b2, Wp1, bp1, Wp2, bp2):
    in_maps = _make_in_maps(x, W1, b1, W2, b2, Wp1, bp1, Wp2, bp2)
    res = _run(in_maps, trace=False)
    out = np.empty((B, B), dtype=np.float32)
    for c in range(NCORES):
        blk = np.asarray(res.results[c]["out"], dtype=np.float32)
        # device block row r*BLK.. maps rows (g + BLK*b); device row order is
        # [g + 16b] = natural order, so rows are already 0..63
        out[c * RPC : (c + 1) * RPC, :] = np.roll(blk, c * RPC, axis=1)
    return out


# revision 20
# speedup vs baseline: 4.2412x; 1.1140x over previous
"""Trainium2 Bass kernel for ExemplarGNN2AdjModel (gnn_message_passing).

Math:
  h  = relu(relu(x@W1+b1)@W2+b2)                      # [512,128] node encoder
  scores[i,j] = Wp2 . relu(Wp1a.h_i + Wp1b.h_j + Wp1c.|h_i-h_j| + bp1) + bp2

Device algorithm (per core, SPMD over 8 cores; core c handles 64 rows of i):
  - Each core receives x pre-rolled by c*64 rows and pre-transposed (xT), so the
    identical program computes rows [c*64, c*64+64) in its local (rolled) node
    order; the host un-rolls the output columns afterwards.
  - |h_i-h_j| = h_i + h_j - 2*min(h_i,h_j): the h_i term is folded into the
    per-i bias matrix (wp1a += w3), the h_j term into the B matmul
    (w2p += w3), and the per-pair part is -2*w3^T min(h_i, h_j).
  - Encoder runs on-device in bf16 (all 512 nodes, replicated per core),
    fp32 PSUM accumulation, fp32 biases.  A2 = wp1a^T h + bp1 precomputed once.
  - The 64 rows are processed in 16 groups of 4, one row from each of the four
    16-row output blocks (i, i+16, i+32, i+48).  Per group:
      d_r  = min(h, h_r)                  DVE tensor_scalar, 1 group lookahead
      P_r  = w2p^T h + w3^T d_r           8 PE matmuls (acc pairs 4 slots apart
                                          so the same-bank accumulate never
                                          stalls on the PSUM drain)
      hid_r = relu(P_r + A2[:,r])         blocks 0-2 on ACT (bias), block 3 on
                                          DVE (tensor_scalar add,max)
      out[16b+i,:] += embW_r^T hid_r      4 col-tiled PE matmuls to PSUM
                                          partitions 0-15/32-47/64-79/96-111 of
                                          ONE bank -- disjoint col_grp strips
                                          run concurrently (~1 slot for all 4)
    The out matmuls of group g are issued in group g+2 so the in-order PE never
    waits on a relu.
  - embW_r = embbuf[:, 15-i : 31-i]: a sliding window over a 31-column zero
    buffer with Wp2 at column 15 puts Wp2 exactly in stationary column i.
  - Startup: xtp is DMA'd in 5 k-chunks with doorbells spread across the sync/
    gpsimd/vector queues (doorbells cost ~600ns each and serialize per queue);
    encoder matmuls start as chunks land; small dummy matmuls cover the DMA
    window so the PE HAM clock-gate is warm (2.4 GHz) for the steady state.
  - Output: one bias add (bp2) over the four block slices + 4 parallel DMAs.
"""

import numpy as np
import ml_dtypes

B = 512
IN_DIM = 595
HID = 128
NCORES = 8
RPC = B // NCORES  # rows per core = 64
NBLK = 4           # output col-tile blocks
BLK = RPC // NBLK  # 16 rows per block
DEFER_G = 2        # groups between producing hid and its out matmul
LOOKA_G = 0        # extra groups of min lookahead beyond the next group
N_WARM_MM = 7      # dummy matmuls to warm the PE HAM clock during input DMAs
WARM_N = 256       # free dim of warm matmuls

# in_dim k-tiles for the first encoder matmul (contraction over 595)
KT = [(0, 128), (128, 256), (256, 384), (384, 512), (512, 595)]

_PROGRAM_CACHE = {}


def _build_program():
    import concourse.mybir as mybir
    import concourse.tile as tile
    from concourse import bacc

    f32 = mybir.dt.float32
    bf16 = mybir.dt.bfloat16
    Act = mybir.ActivationFunctionType
    Alu = mybir.AluOpType

    nc = bacc.Bacc("TRN2", target_bir_lowering=False)

    NKT = len(KT)
    xt_d = nc.dram_tensor("xtp", [HID, NKT * B], bf16, kind="ExternalInput")
    w1_d = nc.dram_tensor("w1p", [HID, NKT * HID], bf16, kind="ExternalInput")
    wpack_d = nc.dram_tensor("wpack", [HID, 4 * HID], bf16, kind="ExternalInput")
    bias_d = nc.dram_tensor("biases", [HID, 4], f32, kind="ExternalInput")
    wp2_d = nc.dram_tensor("wp2col", [HID, 1], bf16, kind="ExternalInput")
    out_d = nc.dram_tensor("out", [RPC, B], f32, kind="ExternalOutput")

    with tile.TileContext(nc) as tc:
        with (
            tc.tile_pool(name="consts", bufs=1) as consts,
            tc.tile_pool(name="setup", bufs=1) as setup,
            tc.tile_pool(name="dwork", bufs=9) as dwork,
            tc.tile_pool(name="hwork", bufs=13) as hwork,
            tc.tile_pool(name="penc", bufs=1, space="PSUM") as penc,
            tc.tile_pool(name="ppair", bufs=7, space="PSUM") as ppair,
        ):
            # ---- input loads first: doorbells cost ~600ns each and serialize
            # per queue, so spread the xtp chunks across three idle queues.
            xt_all = consts.tile([HID, NKT * B], bf16)
            w1_all = consts.tile([HID, NKT * HID], bf16)
            biases = consts.tile([HID, 4], f32)
            wpack = consts.tile([HID, 4 * HID], bf16)
            # earliest-needed first; k-chunks split across sync/gpsimd queues
            nc.scalar.dma_start(out=w1_all, in_=w1_d[:, :])
            qeng = [nc.sync, nc.gpsimd, nc.sync, nc.gpsimd, nc.sync]
            for k in range(NKT):
                qeng[k].dma_start(
                    out=xt_all[:, k * B : (k + 1) * B],
                    in_=xt_d[:, k * B : (k + 1) * B],
                )
            nc.scalar.dma_start(out=biases, in_=bias_d[:, :])
            nc.scalar.dma_start(out=wpack, in_=wpack_d[:, :])

            # ---- PE warm-up over the DMA window (HAM ramps to 2.4 GHz)
            scratch = setup.tile([HID, B], bf16)
            nc.vector.memset(scratch, 0.0)
            scratch1 = setup.tile([HID, 1], f32)
            nc.scalar.activation(scratch1, scratch[:, 0:1], Act.Relu)
            warmp = penc.tile([HID, B], f32, name="warmp", tag="encp")

            def warm_mm(n, w=WARM_N):
                for _ in range(n):
                    nc.tensor.matmul(
                        warmp[:, 0:w], lhsT=scratch[:, 0:HID], rhs=scratch[:, 0:w],
                        start=True, stop=True, skip_group_check=True,
                    )

            warm_mm(N_WARM_MM)

            # sliding-window Wp2 buffer: zeros with Wp2 at column BLK-1; the
            # out matmul for block-row i uses embbuf[:, BLK-1-i+c] == Wp2 iff
            # c == i.
            embbuf = consts.tile([HID, 2 * BLK - 1], bf16)
            nc.vector.memset(embbuf, 0.0)
            nc.gpsimd.dma_start(out=embbuf[:, BLK - 1 : BLK], in_=wp2_d[:, :])

            xt_sb = [xt_all[:, k * B : (k + 1) * B] for k in range(NKT)]
            w1_sb = [w1_all[:, k * HID : (k + 1) * HID] for k in range(NKT)]
            w2_sb = wpack[:, 0 * HID : 1 * HID]
            wp1a_sb = wpack[:, 1 * HID : 2 * HID]
            w2p_sb = wpack[:, 2 * HID : 3 * HID]
            w3_sb = wpack[:, 3 * HID : 4 * HID]
            b1_sb = biases[:, 0:1]
            b2_sb = biases[:, 1:2]
            bp1_sb = biases[:, 2:3]
            bp2_sb = biases[:, 3:4]

            # ---- encoder: h1 = relu(W1^T xT + b1), hT = relu(W2^T h1 + b2) ----
            h1p = penc.tile([HID, B], f32, name="encp", tag="encp")
            for k in range(len(KT)):
                nc.tensor.matmul(
                    h1p, lhsT=w1_sb[k], rhs=xt_sb[k],
                    start=(k == 0), stop=(k == len(KT) - 1),
                )
            warm_mm(2)  # bridge PE over relu1
            # encoder relus split ACT/DVE halves to halve the serial chain
            HB2 = B // 2
            h1bf = setup.tile([HID, B], bf16)
            nc.scalar.activation(h1bf[:, 0:HB2], h1p[:, 0:HB2], Act.Relu, bias=b1_sb)
            nc.vector.tensor_scalar(
                h1bf[:, HB2:B], h1p[:, HB2:B], b1_sb, 0.0, Alu.add, Alu.max
            )

            h2p = penc.tile([HID, B], f32, name="encp2", tag="encp")
            nc.tensor.matmul(h2p, lhsT=w2_sb, rhs=h1bf, start=True, stop=True)
            warm_mm(2)  # bridge PE over relu2
            hbf = setup.tile([HID, B], bf16)
            nc.scalar.activation(hbf[:, 0:HB2], h2p[:, 0:HB2], Act.Relu, bias=b2_sb)
            nc.vector.tensor_scalar(
                hbf[:, HB2:B], h2p[:, HB2:B], b2_sb, 0.0, Alu.add, Alu.max
            )
            # hT fp32 is the per-row scalar operand of the min (tensor_scalar
            # scalars must be fp32); deriving it from hbf instead of h2p avoids
            # a second serialized read of the h2p PSUM bank
            hT = setup.tile([HID, B], f32)
            nc.vector.tensor_copy(hT, hbf)

            def emit_min(j, dtiles):
                if j in dtiles or j >= RPC:
                    return
                d = dwork.tile([HID, B], bf16, name="dtile")
                nc.vector.tensor_scalar(d, hbf, hT[:, j : j + 1], None, Alu.min)
                dtiles[j] = d

            # ---- A2 = wp1a^T h + bp1  (per-i relu bias columns) ----
            a2p = penc.tile([HID, B], f32, name="encp3", tag="encp")
            nc.tensor.matmul(a2p, lhsT=wp1a_sb, rhs=hbf, start=True, stop=True)
            a2 = setup.tile([HID, B], f32)
            nc.scalar.activation(a2, a2p, Act.Identity, bias=bp1_sb)

            # out accumulation in two phases of 8 groups each, reusing ONE
            # PSUM bank (the encoder bank): phase p group g writes partition
            # 32b + (g - 8p) of block b's col_grp strip; the phase-0 flush
            # (copy+bp2 add, 4 strip DMAs) overlaps the phase-1 compute.  The
            # 4 blocks hit disjoint col_grp strips of the PE array and their
            # out matmuls run concurrently.
            PH = BLK // 2  # 8 groups per phase
            outp = penc.tile([HID, B], f32, name="outp", tag="encp")

            def rows_of(g):
                return [g + BLK * b for b in range(NBLK)] if 0 <= g < BLK else []

            dtiles = {}
            pending = {}

            def emit_outs(g):
                go = g % PH
                for b in range(NBLK):
                    r = g + BLK * b
                    hid_r = pending.pop(r)
                    nc.tensor.matmul(
                        outp[32 * b : 32 * b + PH, :],
                        lhsT=embbuf[:, BLK - 1 - go : BLK - 1 - go + PH],
                        rhs=hid_r,
                        start=(go == 0), stop=(go == PH - 1),
                        skip_group_check=True,
                        tile_position=(0, 32 * b),
                    )

            def emit_flush(p):
                # copy+bp2-add PSUM -> SBUF (split DVE/ACT), 4 strip DMAs
                o = setup.tile([HID, B], f32, name=f"outs{p}")
                nc.vector.tensor_scalar(
                    o[:, 0:HB2], outp[:, 0:HB2], bp2_sb, None, Alu.add
                )
                nc.scalar.activation(
                    o[:, HB2:B], outp[:, HB2:B], Act.Identity, bias=bp2_sb
                )
                fq = [nc.sync, nc.gpsimd, nc.sync, nc.gpsimd]
                for b in range(NBLK):
                    fq[b].dma_start(
                        out=out_d[BLK * b + PH * p : BLK * b + PH * (p + 1), :],
                        in_=o[32 * b : 32 * b + PH, :],
                    )

            # prime the min pipeline
            for g0 in range(LOOKA_G + 1):
                for r in rows_of(g0):
                    emit_min(r, dtiles)

            # ---- pairwise main loop: 16 groups of 4 rows ----
            for g in range(BLK):
                rows = rows_of(g)
                for r in rows_of(g + LOOKA_G + 1):
                    emit_min(r, dtiles)
                # deferred out matmuls (4 col-tiled, concurrent)
                if g - DEFER_G >= 0:
                    emit_outs(g - DEFER_G)
                    if g - DEFER_G == PH - 1:
                        emit_flush(0)
                pps = []
                for r in rows:
                    pp = ppair.tile([HID, B], f32, name="pp")
                    nc.tensor.matmul(
                        pp, lhsT=w2p_sb, rhs=hbf,
                        start=True, stop=False, skip_group_check=True,
                    )
                    pps.append(pp)
                for r, pp in zip(rows, pps):
                    nc.tensor.matmul(
                        pp, lhsT=w3_sb, rhs=dtiles.pop(r),
                        start=False, stop=True, skip_group_check=True,
                    )
                # relus: blocks 0-2 on ACT, block 3 on DVE
                for bi, (r, pp) in enumerate(zip(rows, pps)):
                    hid = hwork.tile([HID, B], bf16, name="hid")
                    if bi < 3:
                        nc.scalar.activation(
                            hid, pp, Act.Relu, bias=a2[:, r : r + 1]
                        )
                    else:
                        nc.vector.tensor_scalar(
                            hid, pp, a2[:, r : r + 1], 0.0, Alu.add, Alu.max
                        )
                    pending[r] = hid
            for g in range(BLK - DEFER_G, BLK):
                emit_outs(g)
            emit_flush(1)

    nc.finalize()
    return nc


def _get_program():
    if "nc" not in _PROGRAM_CACHE:
        _PROGRAM_CACHE["nc"] = _build_program()
    return _PROGRAM_CACHE["nc"]


def _make_in_maps(x, W1, b1, W2, b2, Wp1, bp1, Wp2, bp2):
    bf16 = ml_dtypes.bfloat16
    f32 = np.float32
    x = np.asarray(x, dtype=f32)
    W1 = np.asarray(W1, dtype=f32)
    W2 = np.asarray(W2, dtype=f32)
    Wp1 = np.asarray(Wp1, dtype=f32)
    Wp2 = np.asarray(Wp2, dtype=f32).reshape(HID, 1)
    b1c = np.ascontiguousarray(np.asarray(b1, dtype=f32).reshape(HID, 1))
    b2c = np.ascontiguousarray(np.asarray(b2, dtype=f32).reshape(HID, 1))
    bp1c = np.ascontiguousarray(np.asarray(bp1, dtype=f32).reshape(HID, 1))

    # |h_i - h_j| = h_i + h_j - 2*min(h_i, h_j) folds (see module docstring)
    w3f = Wp1[2 * HID : 3 * HID, :]
    wp1a = Wp1[0:HID, :] + w3f
    w2p = Wp1[HID : 2 * HID, :] + w3f
    w3 = -2.0 * w3f

    NKT = len(KT)
    KPAD = NKT * HID  # 640: in_dim padded so every k-tile is 128 partitions

    # packed weights [w2 | wp1a | w2p | w3] and biases [b1 | b2 | bp1 | bp2]
    wpack = np.concatenate([W2, wp1a, w2p, w3], axis=1).astype(bf16)
    biases = np.zeros((HID, 4), dtype=f32)
    biases[:, 0:1] = b1c
    biases[:, 1:2] = b2c
    biases[:, 2:3] = bp1c
    biases[:, 3] = np.asarray(bp2, dtype=f32).reshape(-1)[0]

    wp2col = np.ascontiguousarray(Wp2).astype(bf16)

    # w1 padded to [640, 128], viewed as [128, 5*128]
    w1_pad = np.zeros((KPAD, HID), dtype=f32)
    w1_pad[:IN_DIM] = np.asarray(W1, dtype=f32)
    w1p = np.ascontiguousarray(
        w1_pad.reshape(NKT, HID, HID).transpose(1, 0, 2).reshape(HID, NKT * HID)
    ).astype(bf16)

    shared = dict(w1p=w1p, wpack=wpack, biases=biases, wp2col=wp2col)
    in_maps = []
    for c in range(NCORES):
        xr = np.roll(x, -c * RPC, axis=0)
        xt_pad = np.zeros((KPAD, B), dtype=f32)
        xt_pad[:IN_DIM] = xr.T
        xtp = np.ascontiguousarray(
            xt_pad.reshape(NKT, HID, B).transpose(1, 0, 2).reshape(HID, NKT * B)
        ).astype(bf16)
        m = dict(shared)
        m["xtp"] = xtp
        in_maps.append(m)
    return in_maps


def _run(in_maps, trace=False):
    from concourse.bass_utils import run_bass_kernel_spmd

    nc = _get_program()
    return run_bass_kernel_spmd(
        nc, in_maps, core_ids=list(range(NCORES)), trace=trace
    )


def kernel(x, W1, b1, W2, b2, Wp1, bp1, Wp2, bp2):
    in_maps = _make_in_maps(x, W1, b1, W2, b2, Wp1, bp1, Wp2, bp2)
    res = _run(in_maps, trace=False)
    out = np.empty((B, B), dtype=np.float32)
    for c in range(NCORES):
        blk = np.asarray(res.results[c]["out"], dtype=np.float32)
        # device block row r*BLK.. maps rows (g + BLK*b); device row order is
        # [g + 16b] = natural order, so rows are already 0..63
        out[c * RPC : (c + 1) * RPC, :] = np.roll(blk, c * RPC, axis=1)
    return out


# revision 25
# speedup vs baseline: 4.4004x; 1.0375x over previous
"""Trainium2 Bass kernel for ExemplarGNN2AdjModel (gnn_message_passing).

Math:
  h  = relu(relu(x@W1+b1)@W2+b2)                      # [512,128] node encoder
  scores[i,j] = Wp2 . relu(Wp1a.h_i + Wp1b.h_j + Wp1c.|h_i-h_j| + bp1) + bp2

Device algorithm (per core, SPMD over 8 cores; core c handles 64 rows of i):
  - Each core receives x pre-rolled by c*64 rows and pre-transposed (xT), so the
    identical program computes rows [c*64, c*64+64) in its local (rolled) node
    order; the host un-rolls the output columns afterwards.
  - |h_i-h_j| = h_i + h_j - 2*min(h_i,h_j): the h_i term is folded into the
    per-i bias matrix (wp1a += w3), the h_j term into the B matmul
    (w2p += w3), and the per-pair part is -2*w3^T min(h_i, h_j).
  - Encoder runs on-device in bf16 (all 512 nodes, replicated per core),
    fp32 PSUM accumulation, fp32 biases.  A2 = wp1a^T h + bp1 precomputed once.
  - The 64 rows are processed in 16 groups of 4, one row from each of the four
    16-row output blocks (i, i+16, i+32, i+48).  Per group:
      d_r  = min(h, h_r)                  DVE tensor_scalar, 1 group lookahead
      P_r  = w2p^T h + w3^T d_r           8 PE matmuls (acc pairs 4 slots apart
                                          so the same-bank accumulate never
                                          stalls on the PSUM drain)
      hid_r = relu(P_r + A2[:,r])         blocks 0-2 on ACT (bias), block 3 on
                                          DVE (tensor_scalar add,max)
      out[16b+i,:] += embW_r^T hid_r      4 col-tiled PE matmuls to PSUM
                                          partitions 0-15/32-47/64-79/96-111 of
                                          ONE bank -- disjoint col_grp strips
                                          run concurrently (~1 slot for all 4)
    The out matmuls of group g are issued in group g+2 so the in-order PE never
    waits on a relu.
  - embW_r = embbuf[:, 15-i : 31-i]: a sliding window over a 31-column zero
    buffer with Wp2 at column 15 puts Wp2 exactly in stationary column i.
  - Startup: xtp is DMA'd in 5 k-chunks with doorbells spread across the sync/
    gpsimd/vector queues (doorbells cost ~600ns each and serialize per queue);
    encoder matmuls start as chunks land; small dummy matmuls cover the DMA
    window so the PE HAM clock-gate is warm (2.4 GHz) for the steady state.
  - Output: one bias add (bp2) over the four block slices + 4 parallel DMAs.
"""

import numpy as np
import ml_dtypes

B = 512
IN_DIM = 595
HID = 128
NCORES = 8
RPC = B // NCORES  # rows per core = 64
NBLK = 4           # output col-tile blocks
BLK = RPC // NBLK  # 16 rows per block
DEFER_G = 2        # groups between producing hid and its out matmul
LOOKA_G = 0        # extra groups of min lookahead beyond the next group
N_WARM_MM = 7      # dummy matmuls to warm the PE HAM clock during input DMAs
WARM_N = 256       # free dim of warm matmuls

# in_dim k-tiles for the first encoder matmul (contraction over 595)
KT = [(0, 128), (128, 256), (256, 384), (384, 512), (512, 595)]

_PROGRAM_CACHE = {}


def _build_program():
    import concourse.mybir as mybir
    import concourse.tile as tile
    from concourse import bacc

    f32 = mybir.dt.float32
    bf16 = mybir.dt.bfloat16
    Act = mybir.ActivationFunctionType
    Alu = mybir.AluOpType

    nc = bacc.Bacc("TRN2", target_bir_lowering=False)

    NKT = len(KT)
    xt_d = nc.dram_tensor("xtp", [HID, NKT * B], bf16, kind="ExternalInput")
    w1_d = nc.dram_tensor("w1p", [HID, NKT * HID], bf16, kind="ExternalInput")
    wpack_d = nc.dram_tensor("wpack", [HID, 4 * HID], bf16, kind="ExternalInput")
    bias_d = nc.dram_tensor("biases", [HID, 4], f32, kind="ExternalInput")
    wp2_d = nc.dram_tensor("wp2col", [HID, 1], bf16, kind="ExternalInput")
    out_d = nc.dram_tensor("out", [RPC, B], f32, kind="ExternalOutput")

    with tile.TileContext(nc) as tc:
        with (
            tc.tile_pool(name="consts", bufs=1) as consts,
            tc.tile_pool(name="setup", bufs=1) as setup,
            tc.tile_pool(name="dwork", bufs=9) as dwork,
            tc.tile_pool(name="hwork", bufs=13) as hwork,
            tc.tile_pool(name="penc", bufs=1, space="PSUM") as penc,
            tc.tile_pool(name="ppair", bufs=7, space="PSUM") as ppair,
        ):
            # ---- input loads first: doorbells cost ~600ns each and serialize
            # per queue, so spread the xtp chunks across three idle queues.
            xt_all = consts.tile([HID, NKT * B], bf16)
            w1_all = consts.tile([HID, NKT * HID], bf16)
            biases = consts.tile([HID, 4], f32)
            wpack = consts.tile([HID, 4 * HID], bf16)
            # earliest-needed first; k-chunks split across sync/gpsimd queues
            nc.scalar.dma_start(out=w1_all, in_=w1_d[:, :])
            qeng = [nc.sync, nc.gpsimd, nc.sync, nc.gpsimd, nc.sync]
            for k in range(NKT):
                qeng[k].dma_start(
                    out=xt_all[:, k * B : (k + 1) * B],
                    in_=xt_d[:, k * B : (k + 1) * B],
                )
            nc.scalar.dma_start(out=biases, in_=bias_d[:, :])
            nc.scalar.dma_start(out=wpack, in_=wpack_d[:, :])

            # ---- PE warm-up over the DMA window (HAM ramps to 2.4 GHz)
            scratch = setup.tile([HID, B], bf16)
            nc.vector.memset(scratch, 0.0)
            scratch1 = setup.tile([HID, 1], f32)
            nc.scalar.activation(scratch1, scratch[:, 0:1], Act.Relu)

            def warm_mm(n, w=WARM_N):
                # dummy matmuls keep the PE busy (HAM clock-gate stays at
                # 2.4 GHz) across DMA-wait and relu-wait gaps; they use ppair
                # banks so they never touch the encoder/out accumulator bank
                for _ in range(n):
                    wp = ppair.tile([HID, B], f32, name="pp")
                    nc.tensor.matmul(
                        wp[:, 0:w], lhsT=scratch[:, 0:HID], rhs=scratch[:, 0:w],
                        start=True, stop=True, skip_group_check=True,
                    )

            warm_mm(N_WARM_MM)

            # sliding-window Wp2 buffer: zeros with Wp2 at column BLK-1; the
            # out matmul for block-row i uses embbuf[:, BLK-1-i+c] == Wp2 iff
            # c == i.
            embbuf = consts.tile([HID, 2 * BLK - 1], bf16)
            nc.vector.memset(embbuf, 0.0)
            nc.gpsimd.dma_start(out=embbuf[:, BLK - 1 : BLK], in_=wp2_d[:, :])

            xt_sb = [xt_all[:, k * B : (k + 1) * B] for k in range(NKT)]
            w1_sb = [w1_all[:, k * HID : (k + 1) * HID] for k in range(NKT)]
            w2_sb = wpack[:, 0 * HID : 1 * HID]
            wp1a_sb = wpack[:, 1 * HID : 2 * HID]
            w2p_sb = wpack[:, 2 * HID : 3 * HID]
            w3_sb = wpack[:, 3 * HID : 4 * HID]
            b1_sb = biases[:, 0:1]
            b2_sb = biases[:, 1:2]
            bp1_sb = biases[:, 2:3]
            bp2_sb = biases[:, 3:4]

            # ---- encoder: h1 = relu(W1^T xT + b1), hT = relu(W2^T h1 + b2) ----
            h1p = penc.tile([HID, B], f32, name="encp", tag="encp")
            for k in range(len(KT)):
                nc.tensor.matmul(
                    h1p, lhsT=w1_sb[k], rhs=xt_sb[k],
                    start=(k == 0), stop=(k == len(KT) - 1),
                )
                if k > 0:
                    warm_mm(1)  # bridge the DMA-gated gaps between k-chunks
            # encoder relus split ACT/DVE halves to halve the serial chain
            HB2 = B // 2
            h1bf = setup.tile([HID, B], bf16)
            nc.scalar.activation(h1bf[:, 0:HB2], h1p[:, 0:HB2], Act.Relu, bias=b1_sb)
            nc.vector.tensor_scalar(
                h1bf[:, HB2:B], h1p[:, HB2:B], b1_sb, 0.0, Alu.add, Alu.max
            )

            h2p = penc.tile([HID, B], f32, name="encp2", tag="encp")
            nc.tensor.matmul(h2p, lhsT=w2_sb, rhs=h1bf, start=True, stop=True)
            warm_mm(3)  # bridge PE over relu2 + hT
            hbf = setup.tile([HID, B], bf16)
            nc.scalar.activation(hbf[:, 0:HB2], h2p[:, 0:HB2], Act.Relu, bias=b2_sb)
            nc.vector.tensor_scalar(
                hbf[:, HB2:B], h2p[:, HB2:B], b2_sb, 0.0, Alu.add, Alu.max
            )
            # hT fp32 is the per-row scalar operand of the min (tensor_scalar
            # scalars must be fp32); deriving it from hbf instead of h2p avoids
            # a second serialized read of the h2p PSUM bank
            hT = setup.tile([HID, B], f32)
            nc.vector.tensor_copy(hT, hbf)

            def emit_min(j, dtiles):
                if j in dtiles or j >= RPC:
                    return
                d = dwork.tile([HID, B], bf16, name="dtile")
                nc.vector.tensor_scalar(d, hbf, hT[:, j : j + 1], None, Alu.min)
                dtiles[j] = d

            # ---- A2 = wp1a^T h + bp1  (per-i relu bias columns) ----
            a2p = penc.tile([HID, B], f32, name="encp3", tag="encp")
            nc.tensor.matmul(a2p, lhsT=wp1a_sb, rhs=hbf, start=True, stop=True)
            warm_mm(2)  # bridge PE over the first mins
            a2 = setup.tile([HID, B], f32)
            nc.scalar.activation(a2, a2p, Act.Identity, bias=bp1_sb)

            # out accumulation in two phases of 8 groups each, reusing ONE
            # PSUM bank (the encoder bank): phase p group g writes partition
            # 32b + (g - 8p) of block b's col_grp strip; the phase-0 flush
            # (copy+bp2 add, 4 strip DMAs) overlaps the phase-1 compute.  The
            # 4 blocks hit disjoint col_grp strips of the PE array and their
            # out matmuls run concurrently.
            PH = BLK // 2  # 8 groups per phase
            outp = penc.tile([HID, B], f32, name="outp", tag="encp")

            def rows_of(g):
                return [g + BLK * b for b in range(NBLK)] if 0 <= g < BLK else []

            dtiles = {}
            pending = {}

            def emit_outs(g):
                go = g % PH
                for b in range(NBLK):
                    r = g + BLK * b
                    hid_r = pending.pop(r)
                    nc.tensor.matmul(
                        outp[32 * b : 32 * b + PH, :],
                        lhsT=embbuf[:, BLK - 1 - go : BLK - 1 - go + PH],
                        rhs=hid_r,
                        start=(go == 0), stop=(go == PH - 1),
                        skip_group_check=True,
                        tile_position=(0, 32 * b),
                    )

            def emit_flush(p):
                # copy+bp2-add PSUM -> SBUF (split DVE/ACT), 4 strip DMAs
                o = setup.tile([HID, B], f32, name=f"outs{p}")
                nc.vector.tensor_scalar(
                    o[:, 0:HB2], outp[:, 0:HB2], bp2_sb, None, Alu.add
                )
                nc.scalar.activation(
                    o[:, HB2:B], outp[:, HB2:B], Act.Identity, bias=bp2_sb
                )
                fq = [nc.sync, nc.gpsimd, nc.sync, nc.gpsimd]
                for b in range(NBLK):
                    fq[b].dma_start(
                        out=out_d[BLK * b + PH * p : BLK * b + PH * (p + 1), :],
                        in_=o[32 * b : 32 * b + PH, :],
                    )

            # prime the min pipeline
            for g0 in range(LOOKA_G + 1):
                for r in rows_of(g0):
                    emit_min(r, dtiles)

            # ---- pairwise main loop: 16 groups of 4 rows ----
            for g in range(BLK):
                rows = rows_of(g)
                for r in rows_of(g + LOOKA_G + 1):
                    emit_min(r, dtiles)
                # deferred out matmuls (4 col-tiled, concurrent)
                if g - DEFER_G >= 0:
                    emit_outs(g - DEFER_G)
                    if g - DEFER_G == PH - 1:
                        emit_flush(0)
                pps = []
                for r in rows:
                    pp = ppair.tile([HID, B], f32, name="pp")
                    nc.tensor.matmul(
                        pp, lhsT=w2p_sb, rhs=hbf,
                        start=True, stop=False, skip_group_check=True,
                    )
                    pps.append(pp)
                for r, pp in zip(rows, pps):
                    nc.tensor.matmul(
                        pp, lhsT=w3_sb, rhs=dtiles.pop(r),
                        start=False, stop=True, skip_group_check=True,
                    )
                # relus: blocks 0-2 on ACT, block 3 on DVE
                for bi, (r, pp) in enumerate(zip(rows, pps)):
                    hid = hwork.tile([HID, B], bf16, name="hid")
                    if bi < 3:
                        nc.scalar.activation(
                            hid, pp, Act.Relu, bias=a2[:, r : r + 1]
                        )
                    else:
                        nc.vector.tensor_scalar(
                            hid, pp, a2[:, r : r + 1], 0.0, Alu.add, Alu.max
                        )
                    pending[r] = hid
            for g in range(BLK - DEFER_G, BLK):
                emit_outs(g)
            emit_flush(1)

    nc.finalize()
    return nc


def _get_program():
    if "nc" not in _PROGRAM_CACHE:
        _PROGRAM_CACHE["nc"] = _build_program()
    return _PROGRAM_CACHE["nc"]


def _make_in_maps(x, W1, b1, W2, b2, Wp1, bp1, Wp2, bp2):
    bf16 = ml_dtypes.bfloat16
    f32 = np.float32
    x = np.asarray(x, dtype=f32)
    W1 = np.asarray(W1, dtype=f32)
    W2 = np.asarray(W2, dtype=f32)
    Wp1 = np.asarray(Wp1, dtype=f32)
    Wp2 = np.asarray(Wp2, dtype=f32).reshape(HID, 1)
    b1c = np.ascontiguousarray(np.asarray(b1, dtype=f32).reshape(HID, 1))
    b2c = np.ascontiguousarray(np.asarray(b2, dtype=f32).reshape(HID, 1))
    bp1c = np.ascontiguousarray(np.asarray(bp1, dtype=f32).reshape(HID, 1))

    # |h_i - h_j| = h_i + h_j - 2*min(h_i, h_j) folds (see module docstring)
    w3f = Wp1[2 * HID : 3 * HID, :]
    wp1a = Wp1[0:HID, :] + w3f
    w2p = Wp1[HID : 2 * HID, :] + w3f
    w3 = -2.0 * w3f

    NKT = len(KT)
    KPAD = NKT * HID  # 640: in_dim padded so every k-tile is 128 partitions

    # packed weights [w2 | wp1a | w2p | w3] and biases [b1 | b2 | bp1 | bp2]
    wpack = np.concatenate([W2, wp1a, w2p, w3], axis=1).astype(bf16)
    biases = np.zeros((HID, 4), dtype=f32)
    biases[:, 0:1] = b1c
    biases[:, 1:2] = b2c
    biases[:, 2:3] = bp1c
    biases[:, 3] = np.asarray(bp2, dtype=f32).reshape(-1)[0]

    wp2col = np.ascontiguousarray(Wp2).astype(bf16)

    # w1 padded to [640, 128], viewed as [128, 5*128]
    w1_pad = np.zeros((KPAD, HID), dtype=f32)
    w1_pad[:IN_DIM] = np.asarray(W1, dtype=f32)
    w1p = np.ascontiguousarray(
        w1_pad.reshape(NKT, HID, HID).transpose(1, 0, 2).reshape(HID, NKT * HID)
    ).astype(bf16)

    shared = dict(w1p=w1p, wpack=wpack, biases=biases, wp2col=wp2col)
    in_maps = []
    for c in range(NCORES):
        xr = np.roll(x, -c * RPC, axis=0)
        xt_pad = np.zeros((KPAD, B), dtype=f32)
        xt_pad[:IN_DIM] = xr.T
        xtp = np.ascontiguousarray(
            xt_pad.reshape(NKT, HID, B).transpose(1, 0, 2).reshape(HID, NKT * B)
        ).astype(bf16)
        m = dict(shared)
        m["xtp"] = xtp
        in_maps.append(m)
    return in_maps


def _run(in_maps, trace=False):
    from concourse.bass_utils import run_bass_kernel_spmd

    nc = _get_program()
    return run_bass_kernel_spmd(
        nc, in_maps, core_ids=list(range(NCORES)), trace=trace
    )


def kernel(x, W1, b1, W2, b2, Wp1, bp1, Wp2, bp2):
    in_maps = _make_in_maps(x, W1, b1, W2, b2, Wp1, bp1, Wp2, bp2)
    res = _run(in_maps, trace=False)
    out = np.empty((B, B), dtype=np.float32)
    for c in range(NCORES):
        blk = np.asarray(res.results[c]["out"], dtype=np.float32)
        # device block row r*BLK.. maps rows (g + BLK*b); device row order is
        # [g + 16b] = natural order, so rows are already 0..63
        out[c * RPC : (c + 1) * RPC, :] = np.roll(blk, c * RPC, axis=1)
    return out


# revision 26
# speedup vs baseline: 4.4045x; 1.0009x over previous
"""Trainium2 Bass kernel for ExemplarGNN2AdjModel (gnn_message_passing).

Math:
  h  = relu(relu(x@W1+b1)@W2+b2)                      # [512,128] node encoder
  scores[i,j] = Wp2 . relu(Wp1a.h_i + Wp1b.h_j + Wp1c.|h_i-h_j| + bp1) + bp2

Device algorithm (per core, SPMD over 8 cores; core c handles 64 rows of i):
  - Each core receives x pre-rolled by c*64 rows and pre-transposed (xT), so the
    identical program computes rows [c*64, c*64+64) in its local (rolled) node
    order; the host un-rolls the output columns afterwards.
  - |h_i-h_j| = h_i + h_j - 2*min(h_i,h_j): the h_i term is folded into the
    per-i bias matrix (wp1a += w3), the h_j term into the B matmul
    (w2p += w3), and the per-pair part is -2*w3^T min(h_i, h_j).
  - Encoder runs on-device in bf16 (all 512 nodes, replicated per core),
    fp32 PSUM accumulation, fp32 biases.  A2 = wp1a^T h + bp1 precomputed once.
  - The 64 rows are processed in 16 groups of 4, one row from each of the four
    16-row output blocks (i, i+16, i+32, i+48).  Per group:
      d_r  = min(h, h_r)                  DVE tensor_scalar, 1 group lookahead
      P_r  = w2p^T h + w3^T d_r           8 PE matmuls (acc pairs 4 slots apart
                                          so the same-bank accumulate never
                                          stalls on the PSUM drain)
      hid_r = relu(P_r + A2[:,r])         blocks 0-2 on ACT (bias), block 3 on
                                          DVE (tensor_scalar add,max)
      out[16b+i,:] += embW_r^T hid_r      4 col-tiled PE matmuls to PSUM
                                          partitions 0-15/32-47/64-79/96-111 of
                                          ONE bank -- disjoint col_grp strips
                                          run concurrently (~1 slot for all 4)
    The out matmuls of group g are issued in group g+2 so the in-order PE never
    waits on a relu.
  - embW_r = embbuf[:, 15-i : 31-i]: a sliding window over a 31-column zero
    buffer with Wp2 at column 15 puts Wp2 exactly in stationary column i.
  - Startup: xtp is DMA'd in 5 k-chunks with doorbells spread across the sync/
    gpsimd/vector queues (doorbells cost ~600ns each and serialize per queue);
    encoder matmuls start as chunks land; small dummy matmuls cover the DMA
    window so the PE HAM clock-gate is warm (2.4 GHz) for the steady state.
  - Output: one bias add (bp2) over the four block slices + 4 parallel DMAs.
"""

import numpy as np
import ml_dtypes

B = 512
IN_DIM = 595
HID = 128
NCORES = 8
RPC = B // NCORES  # rows per core = 64
NBLK = 4           # output col-tile blocks
BLK = RPC // NBLK  # 16 rows per block
DEFER_G = 2        # groups between producing hid and its out matmul
LOOKA_G = 0        # extra groups of min lookahead beyond the next group
N_WARM_MM = 7      # dummy matmuls to warm the PE HAM clock during input DMAs
WARM_N = 256       # free dim of warm matmuls

# in_dim k-tiles for the first encoder matmul (contraction over 595)
KT = [(0, 128), (128, 256), (256, 384), (384, 512), (512, 595)]

_PROGRAM_CACHE = {}


def _build_program():
    import concourse.mybir as mybir
    import concourse.tile as tile
    from concourse import bacc

    f32 = mybir.dt.float32
    bf16 = mybir.dt.bfloat16
    Act = mybir.ActivationFunctionType
    Alu = mybir.AluOpType

    nc = bacc.Bacc("TRN2", target_bir_lowering=False)

    NKT = len(KT)
    xt_d = nc.dram_tensor("xtp", [HID, NKT * B], bf16, kind="ExternalInput")
    w1_d = nc.dram_tensor("w1p", [HID, NKT * HID], bf16, kind="ExternalInput")
    wpack_d = nc.dram_tensor("wpack", [HID, 4 * HID], bf16, kind="ExternalInput")
    bias_d = nc.dram_tensor("biases", [HID, 4], f32, kind="ExternalInput")
    wp2_d = nc.dram_tensor("wp2col", [HID, 1], bf16, kind="ExternalInput")
    out_d = nc.dram_tensor("out", [RPC, B], f32, kind="ExternalOutput")

    with tile.TileContext(nc) as tc:
        with (
            tc.tile_pool(name="consts", bufs=1) as consts,
            tc.tile_pool(name="setup", bufs=1) as setup,
            tc.tile_pool(name="dwork", bufs=9) as dwork,
            tc.tile_pool(name="hwork", bufs=13) as hwork,
            tc.tile_pool(name="penc", bufs=1, space="PSUM") as penc,
            tc.tile_pool(name="ppair", bufs=7, space="PSUM") as ppair,
        ):
            # ---- input loads first: doorbells cost ~600ns each and serialize
            # per queue, so spread the xtp chunks across three idle queues.
            xt_all = consts.tile([HID, NKT * B], bf16)
            w1_all = consts.tile([HID, NKT * HID], bf16)
            biases = consts.tile([HID, 4], f32)
            wpack = consts.tile([HID, 4 * HID], bf16)
            # earliest-needed first; k-chunks split across sync/gpsimd queues
            nc.scalar.dma_start(out=w1_all, in_=w1_d[:, :])
            qeng = [nc.sync, nc.gpsimd, nc.sync, nc.gpsimd, nc.sync]
            for k in range(NKT):
                qeng[k].dma_start(
                    out=xt_all[:, k * B : (k + 1) * B],
                    in_=xt_d[:, k * B : (k + 1) * B],
                )
            nc.scalar.dma_start(out=biases, in_=bias_d[:, :])
            nc.scalar.dma_start(out=wpack, in_=wpack_d[:, :])

            # ---- PE warm-up over the DMA window (HAM ramps to 2.4 GHz)
            scratch = setup.tile([HID, B], bf16)
            nc.vector.memset(scratch, 0.0)
            scratch1 = setup.tile([HID, 1], f32)
            nc.scalar.activation(scratch1, scratch[:, 0:1], Act.Relu)

            def warm_mm(n, w=WARM_N):
                # dummy matmuls keep the PE busy (HAM clock-gate stays at
                # 2.4 GHz) across DMA-wait and relu-wait gaps; they use ppair
                # banks so they never touch the encoder/out accumulator bank
                for _ in range(n):
                    wp = ppair.tile([HID, B], f32, name="pp")
                    nc.tensor.matmul(
                        wp[:, 0:w], lhsT=scratch[:, 0:HID], rhs=scratch[:, 0:w],
                        start=True, stop=True, skip_group_check=True,
                    )

            warm_mm(N_WARM_MM)

            # sliding-window Wp2 buffer: zeros with Wp2 at column BLK-1; the
            # out matmul for block-row i uses embbuf[:, BLK-1-i+c] == Wp2 iff
            # c == i.
            embbuf = consts.tile([HID, 2 * BLK - 1], bf16)
            nc.vector.memset(embbuf, 0.0)
            nc.gpsimd.dma_start(out=embbuf[:, BLK - 1 : BLK], in_=wp2_d[:, :])

            xt_sb = [xt_all[:, k * B : (k + 1) * B] for k in range(NKT)]
            w1_sb = [w1_all[:, k * HID : (k + 1) * HID] for k in range(NKT)]
            w2_sb = wpack[:, 0 * HID : 1 * HID]
            wp1a_sb = wpack[:, 1 * HID : 2 * HID]
            w2p_sb = wpack[:, 2 * HID : 3 * HID]
            w3_sb = wpack[:, 3 * HID : 4 * HID]
            b1_sb = biases[:, 0:1]
            b2_sb = biases[:, 1:2]
            bp1_sb = biases[:, 2:3]
            bp2_sb = biases[:, 3:4]

            # ---- encoder: h1 = relu(W1^T xT + b1), hT = relu(W2^T h1 + b2) ----
            h1p = penc.tile([HID, B], f32, name="encp", tag="encp")
            for k in range(len(KT)):
                nc.tensor.matmul(
                    h1p, lhsT=w1_sb[k], rhs=xt_sb[k],
                    start=(k == 0), stop=(k == len(KT) - 1),
                )
                if k > 0:
                    warm_mm(1)  # bridge the DMA-gated gaps between k-chunks
            # encoder relus split ACT/DVE halves to halve the serial chain
            HB2 = B // 2
            h1bf = setup.tile([HID, B], bf16)
            nc.scalar.activation(h1bf[:, 0:HB2], h1p[:, 0:HB2], Act.Relu, bias=b1_sb)
            nc.vector.tensor_scalar(
                h1bf[:, HB2:B], h1p[:, HB2:B], b1_sb, 0.0, Alu.add, Alu.max
            )

            h2p = penc.tile([HID, B], f32, name="encp2", tag="encp")
            nc.tensor.matmul(h2p, lhsT=w2_sb, rhs=h1bf, start=True, stop=True)
            warm_mm(3)  # bridge PE over relu2 + hT
            hbf = setup.tile([HID, B], bf16)
            nc.scalar.activation(hbf[:, 0:HB2], h2p[:, 0:HB2], Act.Relu, bias=b2_sb)
            nc.vector.tensor_scalar(
                hbf[:, HB2:B], h2p[:, HB2:B], b2_sb, 0.0, Alu.add, Alu.max
            )
            # hT fp32 is the per-row scalar operand of the min (tensor_scalar
            # scalars must be fp32); deriving it from hbf instead of h2p avoids
            # a second serialized read of the h2p PSUM bank
            hT = setup.tile([HID, B], f32)
            nc.vector.tensor_copy(hT, hbf)

            def emit_min(j, dtiles):
                if j in dtiles or j >= RPC:
                    return
                d = dwork.tile([HID, B], bf16, name="dtile")
                nc.vector.tensor_scalar(d, hbf, hT[:, j : j + 1], None, Alu.min)
                dtiles[j] = d

            # ---- A2 = wp1a^T h + bp1  (per-i relu bias columns) ----
            a2p = penc.tile([HID, B], f32, name="encp3", tag="encp")
            nc.tensor.matmul(a2p, lhsT=wp1a_sb, rhs=hbf, start=True, stop=True)
            warm_mm(2)  # bridge PE over the first mins
            a2 = setup.tile([HID, B], f32)
            nc.scalar.activation(a2, a2p, Act.Identity, bias=bp1_sb)

            # out accumulation in two phases of 8 groups each, reusing ONE
            # PSUM bank (the encoder bank): phase p group g writes partition
            # 32b + (g - 8p) of block b's col_grp strip; the phase-0 flush
            # (copy+bp2 add, 4 strip DMAs) overlaps the phase-1 compute.  The
            # 4 blocks hit disjoint col_grp strips of the PE array and their
            # out matmuls run concurrently.
            PH = BLK // 2  # 8 groups per phase
            outp = penc.tile([HID, B], f32, name="outp", tag="encp")

            def rows_of(g):
                return [g + BLK * b for b in range(NBLK)] if 0 <= g < BLK else []

            dtiles = {}
            pending = {}

            def emit_outs(g):
                go = g % PH
                for b in range(NBLK):
                    r = g + BLK * b
                    hid_r = pending.pop(r)
                    nc.tensor.matmul(
                        outp[32 * b : 32 * b + PH, :],
                        lhsT=embbuf[:, BLK - 1 - go : BLK - 1 - go + PH],
                        rhs=hid_r,
                        start=(go == 0), stop=(go == PH - 1),
                        skip_group_check=True,
                        tile_position=(0, 32 * b),
                    )

            def emit_flush(p):
                # copy+bp2-add PSUM -> SBUF split by partition strips (DVE
                # covers blocks 0-1, ACT blocks 2-3) so each strip DMA waits
                # only on its own copy half; doorbells spread over 3 queues
                o = setup.tile([HID, B], f32, name=f"outs{p}")
                nc.vector.tensor_scalar(
                    o[0:64, :], outp[0:64, :], bp2_sb[0:64, :], None, Alu.add
                )
                nc.scalar.activation(
                    o[64:128, :], outp[64:128, :], Act.Identity,
                    bias=bp2_sb[64:128, :],
                )
                fq = [nc.sync, nc.gpsimd, nc.scalar, nc.sync]
                for b in range(NBLK):
                    fq[b].dma_start(
                        out=out_d[BLK * b + PH * p : BLK * b + PH * (p + 1), :],
                        in_=o[32 * b : 32 * b + PH, :],
                    )

            # prime the min pipeline
            for g0 in range(LOOKA_G + 1):
                for r in rows_of(g0):
                    emit_min(r, dtiles)

            # ---- pairwise main loop: 16 groups of 4 rows ----
            for g in range(BLK):
                rows = rows_of(g)
                for r in rows_of(g + LOOKA_G + 1):
                    emit_min(r, dtiles)
                # deferred out matmuls (4 col-tiled, concurrent)
                if g - DEFER_G >= 0:
                    emit_outs(g - DEFER_G)
                    if g - DEFER_G == PH - 1:
                        emit_flush(0)
                pps = []
                for r in rows:
                    pp = ppair.tile([HID, B], f32, name="pp")
                    nc.tensor.matmul(
                        pp, lhsT=w2p_sb, rhs=hbf,
                        start=True, stop=False, skip_group_check=True,
                    )
                    pps.append(pp)
                for r, pp in zip(rows, pps):
                    nc.tensor.matmul(
                        pp, lhsT=w3_sb, rhs=dtiles.pop(r),
                        start=False, stop=True, skip_group_check=True,
                    )
                # relus: blocks 0-2 on ACT, block 3 on DVE
                for bi, (r, pp) in enumerate(zip(rows, pps)):
                    hid = hwork.tile([HID, B], bf16, name="hid")
                    if bi < 3:
                        nc.scalar.activation(
                            hid, pp, Act.Relu, bias=a2[:, r : r + 1]
                        )
                    else:
                        nc.vector.tensor_scalar(
                            hid, pp, a2[:, r : r + 1], 0.0, Alu.add, Alu.max
                        )
                    pending[r] = hid
            for g in range(BLK - DEFER_G, BLK):
                emit_outs(g)
            emit_flush(1)

    nc.finalize()
    return nc


def _get_program():
    if "nc" not in _PROGRAM_CACHE:
        _PROGRAM_CACHE["nc"] = _build_program()
    return _PROGRAM_CACHE["nc"]


def _make_in_maps(x, W1, b1, W2, b2, Wp1, bp1, Wp2, bp2):
    bf16 = ml_dtypes.bfloat16
    f32 = np.float32
    x = np.asarray(x, dtype=f32)
    W1 = np.asarray(W1, dtype=f32)
    W2 = np.asarray(W2, dtype=f32)
    Wp1 = np.asarray(Wp1, dtype=f32)
    Wp2 = np.asarray(Wp2, dtype=f32).reshape(HID, 1)
    b1c = np.ascontiguousarray(np.asarray(b1, dtype=f32).reshape(HID, 1))
    b2c = np.ascontiguousarray(np.asarray(b2, dtype=f32).reshape(HID, 1))
    bp1c = np.ascontiguousarray(np.asarray(bp1, dtype=f32).reshape(HID, 1))

    # |h_i - h_j| = h_i + h_j - 2*min(h_i, h_j) folds (see module docstring)
    w3f = Wp1[2 * HID : 3 * HID, :]
    wp1a = Wp1[0:HID, :] + w3f
    w2p = Wp1[HID : 2 * HID, :] + w3f
    w3 = -2.0 * w3f

    NKT = len(KT)
    KPAD = NKT * HID  # 640: in_dim padded so every k-tile is 128 partitions

    # packed weights [w2 | wp1a | w2p | w3] and biases [b1 | b2 | bp1 | bp2]
    wpack = np.concatenate([W2, wp1a, w2p, w3], axis=1).astype(bf16)
    biases = np.zeros((HID, 4), dtype=f32)
    biases[:, 0:1] = b1c
    biases[:, 1:2] = b2c
    biases[:, 2:3] = bp1c
    biases[:, 3] = np.asarray(bp2, dtype=f32).reshape(-1)[0]

    wp2col = np.ascontiguousarray(Wp2).astype(bf16)

    # w1 padded to [640, 128], viewed as [128, 5*128]
    w1_pad = np.zeros((KPAD, HID), dtype=f32)
    w1_pad[:IN_DIM] = np.asarray(W1, dtype=f32)
    w1p = np.ascontiguousarray(
        w1_pad.reshape(NKT, HID, HID).transpose(1, 0, 2).reshape(HID, NKT * HID)
    ).astype(bf16)

    shared = dict(w1p=w1p, wpack=wpack, biases=biases, wp2col=wp2col)
    in_maps = []
    for c in range(NCORES):
        xr = np.roll(x, -c * RPC, axis=0)
        xt_pad = np.zeros((KPAD, B), dtype=f32)
        xt_pad[:IN_DIM] = xr.T
        xtp = np.ascontiguousarray(
            xt_pad.reshape(NKT, HID, B).transpose(1, 0, 2).reshape(HID, NKT * B)
        ).astype(bf16)
        m = dict(shared)
        m["xtp"] = xtp
        in_maps.append(m)
    return in_maps


def _run(in_maps, trace=False):
    from concourse.bass_utils import run_bass_kernel_spmd

    nc = _get_program()
    return run_bass_kernel_spmd(
        nc, in_maps, core_ids=list(range(NCORES)), trace=trace
    )


def kernel(x, W1, b1, W2, b2, Wp1, bp1, Wp2, bp2):
    in_maps = _make_in_maps(x, W1, b1, W2, b2, Wp1, bp1, Wp2, bp2)
    res = _run(in_maps, trace=False)
    out = np.empty((B, B), dtype=np.float32)
    for c in range(NCORES):
        blk = np.asarray(res.results[c]["out"], dtype=np.float32)
        # device block row r*BLK.. maps rows (g + BLK*b); device row order is
        # [g + 16b] = natural order, so rows are already 0..63
        out[c * RPC : (c + 1) * RPC, :] = np.roll(blk, c * RPC, axis=1)
    return out


# revision 27
# speedup vs baseline: 4.4351x; 1.0069x over previous
"""Trainium2 Bass kernel for ExemplarGNN2AdjModel (gnn_message_passing).

Math:
  h  = relu(relu(x@W1+b1)@W2+b2)                      # [512,128] node encoder
  scores[i,j] = Wp2 . relu(Wp1a.h_i + Wp1b.h_j + Wp1c.|h_i-h_j| + bp1) + bp2

Device algorithm (per core, SPMD over 8 cores; core c handles 64 rows of i):
  - Each core receives x pre-rolled by c*64 rows and pre-transposed (xT), so the
    identical program computes rows [c*64, c*64+64) in its local (rolled) node
    order; the host un-rolls the output columns afterwards.
  - |h_i-h_j| = h_i + h_j - 2*min(h_i,h_j): the h_i term is folded into the
    per-i bias matrix (wp1a += w3), the h_j term into the B matmul
    (w2p += w3), and the per-pair part is -2*w3^T min(h_i, h_j).
  - Encoder runs on-device in bf16 (all 512 nodes, replicated per core),
    fp32 PSUM accumulation, fp32 biases.  A2 = wp1a^T h + bp1 precomputed once.
  - The 64 rows are processed in 16 groups of 4, one row from each of the four
    16-row output blocks (i, i+16, i+32, i+48).  Per group:
      d_r  = min(h, h_r)                  DVE tensor_scalar, 1 group lookahead
      P_r  = w2p^T h + w3^T d_r           8 PE matmuls (acc pairs 4 slots apart
                                          so the same-bank accumulate never
                                          stalls on the PSUM drain)
      hid_r = relu(P_r + A2[:,r])         blocks 0-2 on ACT (bias), block 3 on
                                          DVE (tensor_scalar add,max)
      out[16b+i,:] += embW_r^T hid_r      4 col-tiled PE matmuls to PSUM
                                          partitions 0-15/32-47/64-79/96-111 of
                                          ONE bank -- disjoint col_grp strips
                                          run concurrently (~1 slot for all 4)
    The out matmuls of group g are issued in group g+2 so the in-order PE never
    waits on a relu.
  - embW_r = embbuf[:, 15-i : 31-i]: a sliding window over a 31-column zero
    buffer with Wp2 at column 15 puts Wp2 exactly in stationary column i.
  - Startup: xtp is DMA'd in 5 k-chunks with doorbells spread across the sync/
    gpsimd/vector queues (doorbells cost ~600ns each and serialize per queue);
    encoder matmuls start as chunks land; small dummy matmuls cover the DMA
    window so the PE HAM clock-gate is warm (2.4 GHz) for the steady state.
  - Output: one bias add (bp2) over the four block slices + 4 parallel DMAs.
"""

import numpy as np
import ml_dtypes

B = 512
IN_DIM = 595
HID = 128
NCORES = 8
RPC = B // NCORES  # rows per core = 64
NBLK = 4           # output col-tile blocks
BLK = RPC // NBLK  # 16 rows per block
DEFER_G = 2        # groups between producing hid and its out matmul
LOOKA_G = 0        # extra groups of min lookahead beyond the next group
N_WARM_MM = 7      # dummy matmuls to warm the PE HAM clock during input DMAs
WARM_N = 256       # free dim of warm matmuls

# in_dim k-tiles for the first encoder matmul (contraction over 595)
KT = [(0, 128), (128, 256), (256, 384), (384, 512), (512, 595)]

_PROGRAM_CACHE = {}


def _build_program():
    import concourse.mybir as mybir
    import concourse.tile as tile
    from concourse import bacc

    f32 = mybir.dt.float32
    bf16 = mybir.dt.bfloat16
    Act = mybir.ActivationFunctionType
    Alu = mybir.AluOpType

    nc = bacc.Bacc("TRN2", target_bir_lowering=False)

    NKT = len(KT)
    xt_d = nc.dram_tensor("xtp", [HID, NKT * B], bf16, kind="ExternalInput")
    w1_d = nc.dram_tensor("w1p", [HID, NKT * HID], bf16, kind="ExternalInput")
    wpack_d = nc.dram_tensor("wpack", [HID, 4 * HID], bf16, kind="ExternalInput")
    bias_d = nc.dram_tensor("biases", [HID, 4], f32, kind="ExternalInput")
    wp2_d = nc.dram_tensor("wp2col", [HID, 1], bf16, kind="ExternalInput")
    out_d = nc.dram_tensor("out", [RPC, B], f32, kind="ExternalOutput")

    with tile.TileContext(nc) as tc:
        with (
            tc.tile_pool(name="consts", bufs=1) as consts,
            tc.tile_pool(name="setup", bufs=1) as setup,
            tc.tile_pool(name="dwork", bufs=9) as dwork,
            tc.tile_pool(name="hwork", bufs=13) as hwork,
            tc.tile_pool(name="penc", bufs=1, space="PSUM") as penc,
            tc.tile_pool(name="ppair", bufs=7, space="PSUM") as ppair,
        ):
            # ---- input loads first: doorbells cost ~600ns each and serialize
            # per queue, so spread the xtp chunks across three idle queues.
            xt_all = consts.tile([HID, NKT * B], bf16)
            w1_all = consts.tile([HID, NKT * HID], bf16)
            biases = consts.tile([HID, 4], f32)
            wpack = consts.tile([HID, 4 * HID], bf16)
            # earliest-needed first; k-chunks split across sync/gpsimd queues
            nc.scalar.dma_start(out=w1_all, in_=w1_d[:, :])
            qeng = [nc.sync, nc.gpsimd, nc.sync, nc.gpsimd, nc.sync]
            for k in range(NKT):
                qeng[k].dma_start(
                    out=xt_all[:, k * B : (k + 1) * B],
                    in_=xt_d[:, k * B : (k + 1) * B],
                )
            nc.scalar.dma_start(out=biases, in_=bias_d[:, :])
            nc.scalar.dma_start(out=wpack, in_=wpack_d[:, :])

            # ---- PE warm-up over the DMA window (HAM ramps to 2.4 GHz)
            scratch = setup.tile([HID, B], bf16)
            nc.vector.memset(scratch, 0.0)
            scratch1 = setup.tile([HID, 1], f32)
            nc.scalar.activation(scratch1, scratch[:, 0:1], Act.Relu)

            def warm_mm(n, w=WARM_N):
                # dummy matmuls keep the PE busy (HAM clock-gate stays at
                # 2.4 GHz) across DMA-wait and relu-wait gaps; they use ppair
                # banks so they never touch the encoder/out accumulator bank
                for _ in range(n):
                    wp = ppair.tile([HID, B], f32, name="pp")
                    nc.tensor.matmul(
                        wp[:, 0:w], lhsT=scratch[:, 0:HID], rhs=scratch[:, 0:w],
                        start=True, stop=True, skip_group_check=True,
                    )

            warm_mm(N_WARM_MM)

            # sliding-window Wp2 buffer: zeros with Wp2 at column BLK-1; the
            # out matmul for block-row i uses embbuf[:, BLK-1-i+c] == Wp2 iff
            # c == i.
            embbuf = consts.tile([HID, 2 * BLK - 1], bf16)
            nc.vector.memset(embbuf, 0.0)
            nc.gpsimd.dma_start(out=embbuf[:, BLK - 1 : BLK], in_=wp2_d[:, :])

            xt_sb = [xt_all[:, k * B : (k + 1) * B] for k in range(NKT)]
            w1_sb = [w1_all[:, k * HID : (k + 1) * HID] for k in range(NKT)]
            w2_sb = wpack[:, 0 * HID : 1 * HID]
            wp1a_sb = wpack[:, 1 * HID : 2 * HID]
            w2p_sb = wpack[:, 2 * HID : 3 * HID]
            w3_sb = wpack[:, 3 * HID : 4 * HID]
            b1_sb = biases[:, 0:1]
            b2_sb = biases[:, 1:2]
            bp1_sb = biases[:, 2:3]
            bp2_sb = biases[:, 3:4]

            # ---- encoder: h1 = relu(W1^T xT + b1), hT = relu(W2^T h1 + b2) ----
            h1p = penc.tile([HID, B], f32, name="encp", tag="encp")
            for k in range(len(KT)):
                nc.tensor.matmul(
                    h1p, lhsT=w1_sb[k], rhs=xt_sb[k],
                    start=(k == 0), stop=(k == len(KT) - 1),
                )
                if k > 0:
                    warm_mm(1)  # bridge the DMA-gated gaps between k-chunks
            # encoder relus split ACT/DVE halves to halve the serial chain
            HB2 = B // 2
            h1bf = setup.tile([HID, B], bf16)
            nc.scalar.activation(h1bf[:, 0:HB2], h1p[:, 0:HB2], Act.Relu, bias=b1_sb)
            nc.vector.tensor_scalar(
                h1bf[:, HB2:B], h1p[:, HB2:B], b1_sb, 0.0, Alu.add, Alu.max
            )

            h2p = penc.tile([HID, B], f32, name="encp2", tag="encp")
            nc.tensor.matmul(h2p, lhsT=w2_sb, rhs=h1bf, start=True, stop=True)
            warm_mm(3)  # bridge PE over relu2 + hT
            hbf = setup.tile([HID, B], bf16)
            nc.scalar.activation(hbf[:, 0:HB2], h2p[:, 0:HB2], Act.Relu, bias=b2_sb)
            nc.vector.tensor_scalar(
                hbf[:, HB2:B], h2p[:, HB2:B], b2_sb, 0.0, Alu.add, Alu.max
            )
            # hT fp32 is the per-row scalar operand of the min (tensor_scalar
            # scalars must be fp32); deriving it from hbf instead of h2p avoids
            # a second serialized read of the h2p PSUM bank
            hT = setup.tile([HID, B], f32)
            nc.vector.tensor_copy(hT, hbf)

            def emit_min(j, dtiles):
                if j in dtiles or j >= RPC:
                    return
                d = dwork.tile([HID, B], bf16, name="dtile")
                nc.vector.tensor_scalar(d, hbf, hT[:, j : j + 1], None, Alu.min)
                dtiles[j] = d

            # ---- A2 = wp1a^T h + bp1  (per-i relu bias columns) ----
            a2p = penc.tile([HID, B], f32, name="encp3", tag="encp")
            nc.tensor.matmul(a2p, lhsT=wp1a_sb, rhs=hbf, start=True, stop=True)
            warm_mm(2)  # bridge PE over the first mins
            a2 = setup.tile([HID, B], f32)
            nc.scalar.activation(a2, a2p, Act.Identity, bias=bp1_sb)

            # out accumulation in two phases of 8 groups each, reusing ONE
            # PSUM bank (the encoder bank): phase p group g writes partition
            # 32b + (g - 8p) of block b's col_grp strip; the phase-0 flush
            # (copy+bp2 add, 4 strip DMAs) overlaps the phase-1 compute.  The
            # 4 blocks hit disjoint col_grp strips of the PE array and their
            # out matmuls run concurrently.
            PH = BLK // 2  # 8 groups per phase
            outp = penc.tile([HID, B], f32, name="outp", tag="encp")

            def rows_of(g):
                return [g + BLK * b for b in range(NBLK)] if 0 <= g < BLK else []

            dtiles = {}
            pending = {}

            def emit_outs(g):
                go = g % PH
                for b in range(NBLK):
                    r = g + BLK * b
                    hid_r = pending.pop(r)
                    nc.tensor.matmul(
                        outp[32 * b : 32 * b + PH, :],
                        lhsT=embbuf[:, BLK - 1 - go : BLK - 1 - go + PH],
                        rhs=hid_r,
                        start=(go == 0), stop=(go == PH - 1),
                        skip_group_check=True,
                        tile_position=(0, 32 * b),
                    )

            def emit_flush(p):
                # copy+bp2-add PSUM -> SBUF split by partition strips (DVE
                # covers blocks 0-1, ACT blocks 2-3) so each strip DMA waits
                # only on its own copy half; doorbells spread over 3 queues
                o = setup.tile([HID, B], f32, name=f"outs{p}")
                # one DVE op covers all partitions (cost scales with free dim
                # only); gpsimd doorbells are kept off the tail (its SWDGE
                # drain costs ~2us at kernel end)
                nc.vector.tensor_scalar(o, outp, bp2_sb, None, Alu.add)
                fq = (
                    [nc.sync, nc.gpsimd, nc.scalar, nc.gpsimd]
                    if p == 0
                    else [nc.sync, nc.sync, nc.scalar, nc.scalar]
                )
                for b in range(NBLK):
                    fq[b].dma_start(
                        out=out_d[BLK * b + PH * p : BLK * b + PH * (p + 1), :],
                        in_=o[32 * b : 32 * b + PH, :],
                    )

            # prime the min pipeline
            for g0 in range(LOOKA_G + 1):
                for r in rows_of(g0):
                    emit_min(r, dtiles)

            # ---- pairwise main loop: 16 groups of 4 rows ----
            for g in range(BLK):
                rows = rows_of(g)
                for r in rows_of(g + LOOKA_G + 1):
                    emit_min(r, dtiles)
                # deferred out matmuls (4 col-tiled, concurrent)
                if g - DEFER_G >= 0:
                    emit_outs(g - DEFER_G)
                    if g - DEFER_G == PH - 1:
                        emit_flush(0)
                pps = []
                for r in rows:
                    pp = ppair.tile([HID, B], f32, name="pp")
                    nc.tensor.matmul(
                        pp, lhsT=w2p_sb, rhs=hbf,
                        start=True, stop=False, skip_group_check=True,
                    )
                    pps.append(pp)
                for r, pp in zip(rows, pps):
                    nc.tensor.matmul(
                        pp, lhsT=w3_sb, rhs=dtiles.pop(r),
                        start=False, stop=True, skip_group_check=True,
                    )
                # relus: blocks 0-2 on ACT, block 3 on DVE
                for bi, (r, pp) in enumerate(zip(rows, pps)):
                    hid = hwork.tile([HID, B], bf16, name="hid")
                    if bi < 3:
                        nc.scalar.activation(
                            hid, pp, Act.Relu, bias=a2[:, r : r + 1]
                        )
                    else:
                        nc.vector.tensor_scalar(
                            hid, pp, a2[:, r : r + 1], 0.0, Alu.add, Alu.max
                        )
                    pending[r] = hid
            for g in range(BLK - DEFER_G, BLK):
                emit_outs(g)
            emit_flush(1)

    nc.finalize()
    return nc


def _get_program():
    if "nc" not in _PROGRAM_CACHE:
        _PROGRAM_CACHE["nc"] = _build_program()
    return _PROGRAM_CACHE["nc"]


def _make_in_maps(x, W1, b1, W2, b2, Wp1, bp1, Wp2, bp2):
    bf16 = ml_dtypes.bfloat16
    f32 = np.float32
    x = np.asarray(x, dtype=f32)
    W1 = np.asarray(W1, dtype=f32)
    W2 = np.asarray(W2, dtype=f32)
    Wp1 = np.asarray(Wp1, dtype=f32)
    Wp2 = np.asarray(Wp2, dtype=f32).reshape(HID, 1)
    b1c = np.ascontiguousarray(np.asarray(b1, dtype=f32).reshape(HID, 1))
    b2c = np.ascontiguousarray(np.asarray(b2, dtype=f32).reshape(HID, 1))
    bp1c = np.ascontiguousarray(np.asarray(bp1, dtype=f32).reshape(HID, 1))

    # |h_i - h_j| = h_i + h_j - 2*min(h_i, h_j) folds (see module docstring)
    w3f = Wp1[2 * HID : 3 * HID, :]
    wp1a = Wp1[0:HID, :] + w3f
    w2p = Wp1[HID : 2 * HID, :] + w3f
    w3 = -2.0 * w3f

    NKT = len(KT)
    KPAD = NKT * HID  # 640: in_dim padded so every k-tile is 128 partitions

    # packed weights [w2 | wp1a | w2p | w3] and biases [b1 | b2 | bp1 | bp2]
    wpack = np.concatenate([W2, wp1a, w2p, w3], axis=1).astype(bf16)
    biases = np.zeros((HID, 4), dtype=f32)
    biases[:, 0:1] = b1c
    biases[:, 1:2] = b2c
    biases[:, 2:3] = bp1c
    biases[:, 3] = np.asarray(bp2, dtype=f32).reshape(-1)[0]

    wp2col = np.ascontiguousarray(Wp2).astype(bf16)

    # w1 padded to [640, 128], viewed as [128, 5*128]
    w1_pad = np.zeros((KPAD, HID), dtype=f32)
    w1_pad[:IN_DIM] = np.asarray(W1, dtype=f32)
    w1p = np.ascontiguousarray(
        w1_pad.reshape(NKT, HID, HID).transpose(1, 0, 2).reshape(HID, NKT * HID)
    ).astype(bf16)

    shared = dict(w1p=w1p, wpack=wpack, biases=biases, wp2col=wp2col)
    in_maps = []
    for c in range(NCORES):
        xr = np.roll(x, -c * RPC, axis=0)
        xt_pad = np.zeros((KPAD, B), dtype=f32)
        xt_pad[:IN_DIM] = xr.T
        xtp = np.ascontiguousarray(
            xt_pad.reshape(NKT, HID, B).transpose(1, 0, 2).reshape(HID, NKT * B)
        ).astype(bf16)
        m = dict(shared)
        m["xtp"] = xtp
        in_maps.append(m)
    return in_maps


def _run(in_maps, trace=False):
    from concourse.bass_utils import run_bass_kernel_spmd

    nc = _get_program()
    return run_bass_kernel_spmd(
        nc, in_maps, core_ids=list(range(NCORES)), trace=trace
    )


def kernel(x, W1, b1, W2, b2, Wp1, bp1, Wp2, bp2):
    in_maps = _make_in_maps(x, W1, b1, W2, b2, Wp1, bp1, Wp2, bp2)
    res = _run(in_maps, trace=False)
    out = np.empty((B, B), dtype=np.float32)
    for c in range(NCORES):
        blk = np.asarray(res.results[c]["out"], dtype=np.float32)
        # device block row r*BLK.. maps rows (g + BLK*b); device row order is
        # [g + 16b] = natural order, so rows are already 0..63
        out[c * RPC : (c + 1) * RPC, :] = np.roll(blk, c * RPC, axis=1)
    return out


# revision 28
# speedup vs baseline: 4.4827x; 1.0107x over previous
"""Trainium2 Bass kernel for ExemplarGNN2AdjModel (gnn_message_passing).

Math:
  h  = relu(relu(x@W1+b1)@W2+b2)                      # [512,128] node encoder
  scores[i,j] = Wp2 . relu(Wp1a.h_i + Wp1b.h_j + Wp1c.|h_i-h_j| + bp1) + bp2

Device algorithm (per core, SPMD over 8 cores; core c handles 64 rows of i):
  - Each core receives x pre-rolled by c*64 rows and pre-transposed (xT), so the
    identical program computes rows [c*64, c*64+64) in its local (rolled) node
    order; the host un-rolls the output columns afterwards.
  - |h_i-h_j| = h_i + h_j - 2*min(h_i,h_j): the h_i term is folded into the
    per-i bias matrix (wp1a += w3), the h_j term into the B matmul
    (w2p += w3), and the per-pair part is -2*w3^T min(h_i, h_j).
  - Encoder runs on-device in bf16 (all 512 nodes, replicated per core),
    fp32 PSUM accumulation, fp32 biases.  A2 = wp1a^T h + bp1 precomputed once.
  - The 64 rows are processed in 16 groups of 4, one row from each of the four
    16-row output blocks (i, i+16, i+32, i+48).  Per group:
      d_r  = min(h, h_r)                  DVE tensor_scalar, 1 group lookahead
      P_r  = w2p^T h + w3^T d_r           8 PE matmuls (acc pairs 4 slots apart
                                          so the same-bank accumulate never
                                          stalls on the PSUM drain)
      hid_r = relu(P_r + A2[:,r])         blocks 0-2 on ACT (bias), block 3 on
                                          DVE (tensor_scalar add,max)
      out[16b+i,:] += embW_r^T hid_r      4 col-tiled PE matmuls to PSUM
                                          partitions 0-15/32-47/64-79/96-111 of
                                          ONE bank -- disjoint col_grp strips
                                          run concurrently (~1 slot for all 4)
    The out matmuls of group g are issued in group g+2 so the in-order PE never
    waits on a relu.
  - embW_r = embbuf[:, 15-i : 31-i]: a sliding window over a 31-column zero
    buffer with Wp2 at column 15 puts Wp2 exactly in stationary column i.
  - Startup: xtp is DMA'd in 5 k-chunks with doorbells spread across the sync/
    gpsimd/vector queues (doorbells cost ~600ns each and serialize per queue);
    encoder matmuls start as chunks land; small dummy matmuls cover the DMA
    window so the PE HAM clock-gate is warm (2.4 GHz) for the steady state.
  - Output: one bias add (bp2) over the four block slices + 4 parallel DMAs.
"""

import numpy as np
import ml_dtypes

B = 512
IN_DIM = 595
HID = 128
NCORES = 8
RPC = B // NCORES  # rows per core = 64
NBLK = 4           # output col-tile blocks
BLK = RPC // NBLK  # 16 rows per block
DEFER_G = 2        # groups between producing hid and its out matmul
LOOKA_G = 0        # extra groups of min lookahead beyond the next group
N_WARM_MM = 7      # dummy matmuls to warm the PE HAM clock during input DMAs
WARM_N = 256       # free dim of warm matmuls

# in_dim k-tiles for the first encoder matmul (contraction over 595)
KT = [(0, 128), (128, 256), (256, 384), (384, 512), (512, 595)]

_PROGRAM_CACHE = {}


def _build_program():
    import concourse.mybir as mybir
    import concourse.tile as tile
    from concourse import bacc

    f32 = mybir.dt.float32
    bf16 = mybir.dt.bfloat16
    Act = mybir.ActivationFunctionType
    Alu = mybir.AluOpType

    nc = bacc.Bacc("TRN2", target_bir_lowering=False)

    NKT = len(KT)
    xt_d = nc.dram_tensor("xtp", [HID, NKT * B], bf16, kind="ExternalInput")
    w1_d = nc.dram_tensor("w1p", [HID, NKT * HID], bf16, kind="ExternalInput")
    wpack_d = nc.dram_tensor("wpack", [HID, 4 * HID], bf16, kind="ExternalInput")
    bias_d = nc.dram_tensor("biases", [HID, 4], f32, kind="ExternalInput")
    wp2_d = nc.dram_tensor("wp2col", [HID, 1], bf16, kind="ExternalInput")
    out_d = nc.dram_tensor("out", [RPC, B], f32, kind="ExternalOutput")

    with tile.TileContext(nc) as tc:
        with (
            tc.tile_pool(name="consts", bufs=1) as consts,
            tc.tile_pool(name="setup", bufs=1) as setup,
            tc.tile_pool(name="dwork", bufs=9) as dwork,
            tc.tile_pool(name="hwork", bufs=13) as hwork,
            tc.tile_pool(name="penc", bufs=1, space="PSUM") as penc,
            tc.tile_pool(name="ppair", bufs=7, space="PSUM") as ppair,
        ):
            # ---- input loads first: doorbells cost ~600ns each and serialize
            # per queue, so spread the xtp chunks across three idle queues.
            xt_all = consts.tile([HID, NKT * B], bf16)
            w1_all = consts.tile([HID, NKT * HID], bf16)
            biases = consts.tile([HID, 4], f32)
            wpack = consts.tile([HID, 4 * HID], bf16)
            # earliest-needed first; k-chunks split across sync/gpsimd queues
            nc.scalar.dma_start(out=w1_all, in_=w1_d[:, :])
            qeng = [nc.sync, nc.gpsimd, nc.sync, nc.gpsimd, nc.sync]
            for k in range(NKT):
                qeng[k].dma_start(
                    out=xt_all[:, k * B : (k + 1) * B],
                    in_=xt_d[:, k * B : (k + 1) * B],
                )
            nc.scalar.dma_start(out=biases, in_=bias_d[:, :])
            nc.scalar.dma_start(out=wpack, in_=wpack_d[:, :])

            # ---- PE warm-up over the DMA window (HAM ramps to 2.4 GHz)
            scratch = setup.tile([HID, B], bf16)
            nc.vector.memset(scratch, 0.0)
            scratch1 = setup.tile([HID, 1], f32)
            nc.scalar.activation(scratch1, scratch[:, 0:1], Act.Relu)

            def warm_mm(n, w=WARM_N):
                # dummy matmuls keep the PE busy (HAM clock-gate stays at
                # 2.4 GHz) across DMA-wait and relu-wait gaps; they use ppair
                # banks so they never touch the encoder/out accumulator bank
                for _ in range(n):
                    wp = ppair.tile([HID, B], f32, name="pp")
                    nc.tensor.matmul(
                        wp[:, 0:w], lhsT=scratch[:, 0:HID], rhs=scratch[:, 0:w],
                        start=True, stop=True, skip_group_check=True,
                    )

            warm_mm(N_WARM_MM)

            # sliding-window Wp2 buffer: zeros with Wp2 at column BLK-1; the
            # out matmul for block-row i uses embbuf[:, BLK-1-i+c] == Wp2 iff
            # c == i.
            embbuf = consts.tile([HID, 2 * BLK - 1], bf16)
            nc.vector.memset(embbuf, 0.0)
            nc.gpsimd.dma_start(out=embbuf[:, BLK - 1 : BLK], in_=wp2_d[:, :])

            xt_sb = [xt_all[:, k * B : (k + 1) * B] for k in range(NKT)]
            w1_sb = [w1_all[:, k * HID : (k + 1) * HID] for k in range(NKT)]
            w2_sb = wpack[:, 0 * HID : 1 * HID]
            wp1a_sb = wpack[:, 1 * HID : 2 * HID]
            w2p_sb = wpack[:, 2 * HID : 3 * HID]
            w3_sb = wpack[:, 3 * HID : 4 * HID]
            b1_sb = biases[:, 0:1]
            b2_sb = biases[:, 1:2]
            bp1_sb = biases[:, 2:3]
            bp2_sb = biases[:, 3:4]

            # ---- encoder: h1 = relu(W1^T xT + b1), hT = relu(W2^T h1 + b2) ----
            h1p = penc.tile([HID, B], f32, name="encp", tag="encp")
            for k in range(len(KT)):
                nc.tensor.matmul(
                    h1p, lhsT=w1_sb[k], rhs=xt_sb[k],
                    start=(k == 0), stop=(k == len(KT) - 1),
                )
                if k > 0:
                    warm_mm(1)  # bridge the DMA-gated gaps between k-chunks
            # encoder relus split ACT/DVE halves to halve the serial chain
            HB2 = B // 2
            h1bf = setup.tile([HID, B], bf16)
            nc.scalar.activation(h1bf[:, 0:HB2], h1p[:, 0:HB2], Act.Relu, bias=b1_sb)
            nc.vector.tensor_scalar(
                h1bf[:, HB2:B], h1p[:, HB2:B], b1_sb, 0.0, Alu.add, Alu.max
            )

            # h2 in two half-width matmuls to different PSUM banks so the two
            # relu halves (ACT low half, DVE high half) pipeline behind them
            h2p = penc.tile([HID, HB2], f32, name="encp2", tag="encp")
            nc.tensor.matmul(h2p, lhsT=w2_sb, rhs=h1bf[:, 0:HB2], start=True, stop=True)
            h2pb = ppair.tile([HID, B], f32, name="pp")
            nc.tensor.matmul(
                h2pb[:, 0:HB2], lhsT=w2_sb, rhs=h1bf[:, HB2:B],
                start=True, stop=True, skip_group_check=True,
            )
            warm_mm(2)  # bridge PE over relu2 + hT
            hbf = setup.tile([HID, B], bf16)
            nc.scalar.activation(hbf[:, 0:HB2], h2p, Act.Relu, bias=b2_sb)
            nc.vector.tensor_scalar(
                hbf[:, HB2:B], h2pb[:, 0:HB2], b2_sb, 0.0, Alu.add, Alu.max
            )
            # hT fp32 is the per-row scalar operand of the min (tensor_scalar
            # scalars must be fp32); only the core's 64 local-row columns are
            # ever read, and deriving it from hbf avoids a second serialized
            # read of the h2p PSUM bank
            hT = setup.tile([HID, RPC], f32)
            nc.vector.tensor_copy(hT, hbf[:, 0:RPC])

            def emit_min(j, dtiles):
                if j in dtiles or j >= RPC:
                    return
                d = dwork.tile([HID, B], bf16, name="dtile")
                nc.vector.tensor_scalar(d, hbf, hT[:, j : j + 1], None, Alu.min)
                dtiles[j] = d

            # ---- A2 = wp1a^T h + bp1  (per-i relu bias columns) ----
            a2p = penc.tile([HID, B], f32, name="encp3", tag="encp")
            nc.tensor.matmul(a2p, lhsT=wp1a_sb, rhs=hbf, start=True, stop=True)
            warm_mm(2)  # bridge PE over the first mins
            a2 = setup.tile([HID, B], f32)
            nc.scalar.activation(a2, a2p, Act.Identity, bias=bp1_sb)

            # out accumulation in two phases of 8 groups each, reusing ONE
            # PSUM bank (the encoder bank): phase p group g writes partition
            # 32b + (g - 8p) of block b's col_grp strip; the phase-0 flush
            # (copy+bp2 add, 4 strip DMAs) overlaps the phase-1 compute.  The
            # 4 blocks hit disjoint col_grp strips of the PE array and their
            # out matmuls run concurrently.
            PH = BLK // 2  # 8 groups per phase
            outp = penc.tile([HID, B], f32, name="outp", tag="encp")

            def rows_of(g):
                return [g + BLK * b for b in range(NBLK)] if 0 <= g < BLK else []

            dtiles = {}
            pending = {}

            def emit_outs(g):
                go = g % PH
                for b in range(NBLK):
                    r = g + BLK * b
                    hid_r = pending.pop(r)
                    nc.tensor.matmul(
                        outp[32 * b : 32 * b + PH, :],
                        lhsT=embbuf[:, BLK - 1 - go : BLK - 1 - go + PH],
                        rhs=hid_r,
                        start=(go == 0), stop=(go == PH - 1),
                        skip_group_check=True,
                        tile_position=(0, 32 * b),
                    )

            def emit_flush(p):
                # copy+bp2-add PSUM -> SBUF split by partition strips (DVE
                # covers blocks 0-1, ACT blocks 2-3) so each strip DMA waits
                # only on its own copy half; doorbells spread over 3 queues
                o = setup.tile([HID, B], f32, name=f"outs{p}")
                # one DVE op covers all partitions (cost scales with free dim
                # only); gpsimd doorbells are kept off the tail (its SWDGE
                # drain costs ~2us at kernel end)
                nc.vector.tensor_scalar(o, outp, bp2_sb, None, Alu.add)
                fq = (
                    [nc.sync, nc.gpsimd, nc.scalar, nc.gpsimd]
                    if p == 0
                    else [nc.sync, nc.sync, nc.scalar, nc.scalar]
                )
                for b in range(NBLK):
                    fq[b].dma_start(
                        out=out_d[BLK * b + PH * p : BLK * b + PH * (p + 1), :],
                        in_=o[32 * b : 32 * b + PH, :],
                    )

            # prime the min pipeline
            for g0 in range(LOOKA_G + 1):
                for r in rows_of(g0):
                    emit_min(r, dtiles)

            # ---- pairwise main loop: 16 groups of 4 rows ----
            for g in range(BLK):
                rows = rows_of(g)
                for r in rows_of(g + LOOKA_G + 1):
                    emit_min(r, dtiles)
                # deferred out matmuls (4 col-tiled, concurrent)
                if g - DEFER_G >= 0:
                    emit_outs(g - DEFER_G)
                    if g - DEFER_G == PH - 1:
                        emit_flush(0)
                pps = []
                for r in rows:
                    pp = ppair.tile([HID, B], f32, name="pp")
                    nc.tensor.matmul(
                        pp, lhsT=w2p_sb, rhs=hbf,
                        start=True, stop=False, skip_group_check=True,
                    )
                    pps.append(pp)
                for r, pp in zip(rows, pps):
                    nc.tensor.matmul(
                        pp, lhsT=w3_sb, rhs=dtiles.pop(r),
                        start=False, stop=True, skip_group_check=True,
                    )
                # relus: blocks 0-2 on ACT, block 3 on DVE
                for bi, (r, pp) in enumerate(zip(rows, pps)):
                    hid = hwork.tile([HID, B], bf16, name="hid")
                    if bi < 3:
                        nc.scalar.activation(
                            hid, pp, Act.Relu, bias=a2[:, r : r + 1]
                        )
                    else:
                        nc.vector.tensor_scalar(
                            hid, pp, a2[:, r : r + 1], 0.0, Alu.add, Alu.max
                        )
                    pending[r] = hid
            for g in range(BLK - DEFER_G, BLK):
                emit_outs(g)
            emit_flush(1)

    nc.finalize()
    return nc


def _get_program():
    if "nc" not in _PROGRAM_CACHE:
        _PROGRAM_CACHE["nc"] = _build_program()
    return _PROGRAM_CACHE["nc"]


def _make_in_maps(x, W1, b1, W2, b2, Wp1, bp1, Wp2, bp2):
    bf16 = ml_dtypes.bfloat16
    f32 = np.float32
    x = np.asarray(x, dtype=f32)
    W1 = np.asarray(W1, dtype=f32)
    W2 = np.asarray(W2, dtype=f32)
    Wp1 = np.asarray(Wp1, dtype=f32)
    Wp2 = np.asarray(Wp2, dtype=f32).reshape(HID, 1)
    b1c = np.ascontiguousarray(np.asarray(b1, dtype=f32).reshape(HID, 1))
    b2c = np.ascontiguousarray(np.asarray(b2, dtype=f32).reshape(HID, 1))
    bp1c = np.ascontiguousarray(np.asarray(bp1, dtype=f32).reshape(HID, 1))

    # |h_i - h_j| = h_i + h_j - 2*min(h_i, h_j) folds (see module docstring)
    w3f = Wp1[2 * HID : 3 * HID, :]
    wp1a = Wp1[0:HID, :] + w3f
    w2p = Wp1[HID : 2 * HID, :] + w3f
    w3 = -2.0 * w3f

    NKT = len(KT)
    KPAD = NKT * HID  # 640: in_dim padded so every k-tile is 128 partitions

    # packed weights [w2 | wp1a | w2p | w3] and biases [b1 | b2 | bp1 | bp2]
    wpack = np.concatenate([W2, wp1a, w2p, w3], axis=1).astype(bf16)
    biases = np.zeros((HID, 4), dtype=f32)
    biases[:, 0:1] = b1c
    biases[:, 1:2] = b2c
    biases[:, 2:3] = bp1c
    biases[:, 3] = np.asarray(bp2, dtype=f32).reshape(-1)[0]

    wp2col = np.ascontiguousarray(Wp2).astype(bf16)

    # w1 padded to [640, 128], viewed as [128, 5*128]
    w1_pad = np.zeros((KPAD, HID), dtype=f32)
    w1_pad[:IN_DIM] = np.asarray(W1, dtype=f32)
    w1p = np.ascontiguousarray(
        w1_pad.reshape(NKT, HID, HID).transpose(1, 0, 2).reshape(HID, NKT * HID)
    ).astype(bf16)

    shared = dict(w1p=w1p, wpack=wpack, biases=biases, wp2col=wp2col)
    in_maps = []
    for c in range(NCORES):
        xr = np.roll(x, -c * RPC, axis=0)
        xt_pad = np.zeros((KPAD, B), dtype=f32)
        xt_pad[:IN_DIM] = xr.T
        xtp = np.ascontiguousarray(
            xt_pad.reshape(NKT, HID, B).transpose(1, 0, 2).reshape(HID, NKT * B)
        ).astype(bf16)
        m = dict(shared)
        m["xtp"] = xtp
        in_maps.append(m)
    return in_maps


def _run(in_maps, trace=False):
    from concourse.bass_utils import run_bass_kernel_spmd

    nc = _get_program()
    return run_bass_kernel_spmd(
        nc, in_maps, core_ids=list(range(NCORES)), trace=trace
    )


def kernel(x, W1, b1, W2, b2, Wp1, bp1, Wp2, bp2):
    in_maps = _make_in_maps(x, W1, b1, W2, b2, Wp1, bp1, Wp2, bp2)
    res = _run(in_maps, trace=False)
    out = np.empty((B, B), dtype=np.float32)
    for c in range(NCORES):
        blk = np.asarray(res.results[c]["out"], dtype=np.float32)
        # device block row r*BLK.. maps rows (g + BLK*b); device row order is
        # [g + 16b] = natural order, so rows are already 0..63
        out[c * RPC : (c + 1) * RPC, :] = np.roll(blk, c * RPC, axis=1)
    return out
